# revision 35
# baseline (speedup 1.0000x reference)
"""GAT (2-layer graph attention network) on 8 Trainium2 NeuronCores.

Strategy (dst-sharded graph parallel, gather-x-recompute):
  - Nodes are partitioned across the 8 cores (6250 dst nodes each).
  - Host pre-sorts edges (incl. self-loops) by destination, groups them into
    128-dst "windows" and 128-edge "chunks", padding so every core runs an
    identical static program.  Chunks are segregated by source half (int16
    gather-index range), and batched into 2048-edge superchunks.
  - Layer 1 avoids any feature AllGather: each core holds the FULL x as a
    gather table; per superchunk one transposed dma_gather fetches raw x rows
    (512B each) feature-major, and h|alpha_src are recomputed PER EDGE on the
    tensor engine (x_edge^T @ W1aug) - the PE is otherwise idle and this cuts
    gather descriptors/bytes ~3x vs gathering precomputed h rows.
  - alpha_dst needs no gather at all: a_dst lives in an SBUF table indexed by
    (window, slot); the per-chunk one-hot is transposed on the PE and a tiny
    matmul selects per-edge a_dst.
  - Scatter-add into PSUM via one-hot matmuls per 128-dst window; softmax
    denominators from a second matmul against the same one-hot.
  - Layer 2: h2 rows are dst-core-local, so a small AllGather ([N,128] bf16)
    replicates them; per-edge rows gathered as before; a_dst2 via the same
    one-hot-transpose trick; then log_softmax.
"""

import math
from dataclasses import dataclass

import ml_dtypes
import numpy as np

import concourse.bass as bass
import concourse.mybir as mybir
import concourse.tile as tile
from concourse import bacc

BF16 = ml_dtypes.bfloat16
P = 128
NEG_SLOPE = 0.2
HALF = 25000  # src-table split point (int16 index range)


@dataclass(frozen=True)
class Cfg:
    N: int = 50000
    F: int = 256
    H: int = 8
    C: int = 64
    OUT: int = 40
    NC: int = 8
    SCK: int = 32          # chunks per gather superchunk

    @property
    def HC(self):
        return self.H * self.C

    @property
    def NSH(self):
        return self.N // self.NC

    @property
    def NWIN(self):
        return math.ceil(self.NSH / P)

    @property
    def nclass(self):
        return 2 if self.N > 32767 else 1


FULL = Cfg()


# ---------------------------------------------------------------- host side


def _schedule(cfg: Cfg, edge_index: np.ndarray):
    """Sort/pad edges into the common static chunk schedule.

    Returns (nch, TSC_c, eidx, edl):
      nch  : [nclass][NWIN] chunks per (class, window), common to all cores
      eidx : int16 [NC, TSC, 128, SCK*8]  src local-to-table indices
             (idx i of a superchunk at [i % 16 (+16g replicas), i // 16])
      edl  : float32 [NC, TSC, 128, SCK] dst-in-window (pad -1)
    """
    N, NCOR, NSH, SCK = cfg.N, cfg.NC, cfg.NSH, cfg.SCK
    NWIN, NCLS = cfg.NWIN, cfg.nclass
    # self-loops are handled on-chip per window (identity one-hot, local x),
    # so only the real edges enter the gathered chunk stream
    src = np.asarray(edge_index[0]).astype(np.int64)
    dst = np.asarray(edge_index[1]).astype(np.int64)
    core = (dst // NSH).astype(np.int32)
    dl = (dst % NSH).astype(np.int32)
    win = dl // P
    wloc = dl % P
    cls = (src >= HALF).astype(np.int32) if NCLS == 2 else np.zeros_like(core)

    counts = np.zeros((NCOR, NCLS, NWIN), np.int64)
    np.add.at(counts, (core, cls, win), 1)
    nch = np.maximum(np.ceil(counts / P).astype(np.int64).max(axis=0), 1)
    T_c = [int(nch[c].sum()) for c in range(NCLS)]
    TSC_c = [math.ceil(t / SCK) for t in T_c]
    TSC = sum(TSC_c)
    TP_c = [t * SCK for t in TSC_c]

    # slot offset of (class, window) inside its class stream (in edges)
    wstart = np.zeros((NCLS, NWIN + 1), np.int64)
    for c in range(NCLS):
        wstart[c, 1:] = np.cumsum(nch[c])
    wstart *= P

    eidx = np.zeros((NCOR, TSC, P, SCK * 8), np.int16)
    edlA = np.full((NCOR, TSC, P, SCK), -1.0, np.float32)
    sc_base = [0, TSC_c[0]] if NCLS == 2 else [0]

    for k in range(NCOR):
        for c in range(NCLS):
            m = (core == k) & (cls == c)
            s_k = src[m] - (HALF if c == 1 else 0)
            w_k = win[m]
            wl_k = wloc[m]
            order = np.lexsort((s_k, w_k))
            s_k, w_k, wl_k = s_k[order], w_k[order], wl_k[order]
            cnts = counts[k, c]
            offs = np.concatenate([[0], np.cumsum(cnts[:-1])])
            slot = wstart[c, w_k] + (np.arange(len(s_k)) - offs[w_k])
            es = np.zeros(TP_c[c] * P, np.int16)
            el = np.full(TP_c[c] * P, -1.0, np.float32)
            es[slot] = s_k.astype(np.int16)
            el[slot] = wl_k.astype(np.float32)
            # flat i -> idx tile [i%16, i//16] (replicated), edl [i%128, i//128]
            for si in range(TSC_c[c]):
                sc = sc_base[c] + si
                seg_s = es[si * SCK * P:(si + 1) * SCK * P]
                seg_l = el[si * SCK * P:(si + 1) * SCK * P]
                t16s = seg_s.reshape(-1, 16).T  # [16, SCK*8]
                eidx[k, sc, :, :] = np.tile(t16s, (8, 1))
                edlA[k, sc] = seg_l.reshape(SCK, P).T
    nch_py = [[int(x) for x in nch[c]] for c in range(NCLS)]
    return nch_py, TSC_c, eidx, edlA


def _perm(cfg: Cfg):
    p = np.empty(cfg.HC, np.int64)
    for h in range(cfg.H):
        p[np.arange(cfg.C) * cfg.H + h] = h * cfg.C + np.arange(cfg.C)
    return p


def _prep_weights(cfg: Cfg, W1, a_src1, a_dst1, W2, a_src2, a_dst2):
    perm = _perm(cfg)
    H, C, HC, OUT = cfg.H, cfg.C, cfg.HC, cfg.OUT
    Ws1 = np.stack([W1[:, h * C:(h + 1) * C] @ a_src1[h] for h in range(H)], 1)
    Wd1 = np.stack([W1[:, h * C:(h + 1) * C] @ a_dst1[h] for h in range(H)], 1)
    W1aug = np.concatenate([W1[:, perm], Ws1, Wd1], axis=1).astype(BF16)
    w2s = (W2 @ a_src2[0])[:, None]
    w2d = (W2 @ a_dst2[0])[:, None]
    L2PAD = 48 - (OUT + 2)
    W2aug = np.concatenate(
        [W2, w2s, w2d, np.zeros((HC, L2PAD), W2.dtype)], axis=1
    )[perm, :].astype(BF16)
    return W1aug, W2aug


# -------------------------------------------------------------- device side


def _build(cfg: Cfg, nch, TSC_c, b1_nonzero=False, b2_nonzero=False,
           mode="full"):
    N, F, H, C, HC, OUT = cfg.N, cfg.F, cfg.H, cfg.C, cfg.HC, cfg.OUT
    NSH, SCK, NWIN = cfg.NSH, cfg.SCK, cfg.NWIN
    NCLS = cfg.nclass
    TSC = sum(TSC_c)
    KT = math.ceil(F / P)
    BT = math.ceil(HC / P)
    AUG1 = HC + 2 * H
    AUG2 = 48
    ADW = 128                    # hx2 table row stride (elements)
    NT = NWIN
    XPAD = NT * P
    NIDX = SCK * P

    bf = mybir.dt.bfloat16
    f32 = mybir.dt.float32
    i16 = mybir.dt.int16
    AF = mybir.ActivationFunctionType
    OP = mybir.AluOpType

    nc = bacc.Bacc(
        "TRN2", target_bir_lowering=False, debug=False,
        enable_asserts=False, num_devices=cfg.NC,
    )

    xT_t = nc.dram_tensor("xT", [F, XPAD], bf, kind="ExternalInput")
    xtab_t = nc.dram_tensor("xtab", [N, F], bf, kind="ExternalInput")
    w1_t = nc.dram_tensor("W1aug", [F, AUG1], bf, kind="ExternalInput")
    w2_t = nc.dram_tensor("W2aug", [HC, AUG2], bf, kind="ExternalInput")
    iota_t = nc.dram_tensor("iota", [P, P], bf, kind="ExternalInput")
    ident_t = nc.dram_tensor("ident", [P, P], bf, kind="ExternalInput")
    eidx_t = nc.dram_tensor("eidx", [TSC, P, SCK * 8], i16,
                            kind="ExternalInput")
    edl_t = nc.dram_tensor("edl", [TSC, P, SCK], f32, kind="ExternalInput")
    if b1_nonzero:
        b1_t = nc.dram_tensor("b1rep", [P, HC], f32, kind="ExternalInput")
    if b2_nonzero:
        b2_t = nc.dram_tensor("b2rep", [P, OUT], f32, kind="ExternalInput")
    out_t = nc.dram_tensor("out", [NSH, OUT], f32, kind="ExternalOutput")

    # (class, sc, kk) schedule per window: class streams are contiguous
    sc_base = [0, TSC_c[0]] if NCLS == 2 else [0]
    window_chunks = []  # per window: list of (c, sc, kk)
    pos_c = [0] * NCLS
    for w in range(NWIN):
        lst = []
        for c in range(NCLS):
            for _ in range(nch[c][w]):
                j = pos_c[c]
                lst.append((c, sc_base[c] + j // SCK, j % SCK))
                pos_c[c] += 1
        window_chunks.append(lst)
    rows_of = lambda w: min(P, NSH - w * P)
    # class of each sc (for table selection) and window of each (sc, kk)
    sc_cls = [0] * TSC
    if NCLS == 2:
        for s in range(TSC_c[0], TSC):
            sc_cls[s] = 1
    win_of = [[0] * SCK for _ in range(TSC)]
    for w in range(NWIN):
        for (c, sc, kk) in window_chunks[w]:
            win_of[sc][kk] = w

    with tile.TileContext(nc) as tc:
        dram_pool = tc.tile_pool(name="dram", bufs=1, space="DRAM")
        pdr = dram_pool.__enter__()
        shared_as = "Shared" if cfg.NC > 4 else "Local"
        hx2_dram = pdr.tile([NSH, ADW], bf, name="hx2_dram")
        hx2_full = pdr.tile([N, ADW], bf, addr_space=shared_as,
                            name="hx2_full")

        def xtab_src(sc):
            if NCLS == 1 or sc_cls[sc] == 0:
                return xtab_t[0:min(HALF, N), :]
            return xtab_t[HALF:N, :]

        def hx2_src(sc):
            if NCLS == 1 or sc_cls[sc] == 0:
                return hx2_full[0:min(HALF, N), :]
            return hx2_full[HALF:N, :]

        with tc.tile_pool(name="const", bufs=1) as pc:
            iota_sb = pc.tile([P, P], bf, name="iota_sb")
            nc.sync.dma_start(out=iota_sb[:], in_=iota_t[:, :])
            ident_sb = pc.tile([P, P], bf, name="ident_sb")
            nc.sync.dma_start(out=ident_sb[:], in_=ident_t[:, :])
            w1_sb = []
            for kk in range(KT):
                r = min(P, F - kk * P)
                t_ = pc.tile([r, AUG1], bf, name=f"w1_sb{kk}")
                nc.sync.dma_start(out=t_[:], in_=w1_t[kk * P:kk * P + r, :])
                w1_sb.append(t_)
            w2_sb = []
            for b in range(BT):
                r = min(P, HC - b * P)
                t_ = pc.tile([r, AUG2], bf, name=f"w2_sb{b}")
                nc.sync.dma_start(out=t_[:], in_=w2_t[b * P:b * P + r, :])
                w2_sb.append(t_)
            xT_sb = []
            for kk in range(KT):
                r = min(P, F - kk * P)
                t_ = pc.tile([r, XPAD], bf, name=f"xT_sb{kk}")
                nc.sync.dma_start(out=t_[:], in_=xT_t[kk * P:kk * P + r, :])
                xT_sb.append(t_)
            if b1_nonzero:
                b1_sb = pc.tile([P, HC], f32, name="b1_sb")
                nc.sync.dma_start(out=b1_sb[:], in_=b1_t[:, :])
            if b2_nonzero:
                b2_sb = pc.tile([P, OUT], f32, name="b2_sb")
                nc.sync.dma_start(out=b2_sb[:], in_=b2_t[:, :])
            # SBUF a_dst table: [slot, window, head(8) + l2(1)]
            adt_sb = pc.tile([P, NWIN, H + 1], bf, name="adt_sb")
            # SBUF copy of this core's own hx2 rows (for layer-2 self-loops)
            hx2sb = pc.tile([P, NWIN, AUG2], bf, name="hx2sb")

            # ---------------- phase A: a_dst table = (x @ Wd1) per window
            with tc.tile_pool(name="phA_ps", bufs=2, space="PSUM") as pap:
                for t in range(NT):
                    pa = pap.tile([P, 2 * H], f32, tag="pA", name=f"pA{t}")
                    for kk in range(KT):
                        lhsT = xT_sb[kk][:, t * P:(t + 1) * P]
                        nc.tensor.matmul(
                            out=pa[:], lhsT=lhsT, rhs=w1_sb[kk][:, HC:AUG1],
                            start=(kk == 0), stop=(kk == KT - 1))
                    nc.vector.tensor_copy(
                        out=adt_sb[:, t, 0:H], in_=pa[:, H:2 * H])

            # ---------------- phase C: layer-1 edge loop (+ h2 per window)
            with tc.tile_pool(name="phC_st", bufs=2) as pst, \
                 tc.tile_pool(name="phC_ck", bufs=4) as pck, \
                 tc.tile_pool(name="phC_po", bufs=2) as ppo, \
                 tc.tile_pool(name="phC_he", bufs=1, space="PSUM") as phe, \
                 tc.tile_pool(name="phC_ps", bufs=1, space="PSUM") as pps, \
                 tc.tile_pool(name="phC_sp", bufs=1, space="PSUM") as psp, \
                 tc.tile_pool(name="phC_pt", bufs=1, space="PSUM") as ppt:

                sc_cache = {}

                def get_sc(sc):
                    if sc in sc_cache:
                        return sc_cache[sc]
                    six = pst.tile([P, SCK * 8], i16, tag="six",
                                   name=f"six{sc}")
                    nc.sync.dma_start(out=six[:], in_=eidx_t[sc, :, :])
                    edl = pst.tile([P, SCK], f32, tag="edl", name=f"edl{sc}")
                    nc.sync.dma_start(out=edl[:], in_=edl_t[sc, :, :])
                    xg = pst.tile([P, KT, NIDX], bf, tag="xg", name=f"xg{sc}")
                    nc.gpsimd.dma_gather(
                        xg[:], xtab_src(sc), six[:],
                        NIDX, NIDX, F, transpose=True, single_packet=False)
                    ohs = pst.tile([P, SCK, P], bf, tag="ohs", name=f"ohs{sc}")
                    ohT = pst.tile([P, SCK, P], bf, tag="ohT", name=f"ohT{sc}")
                    adsp = psp.tile([P, SCK, H], f32, tag="adsp",
                                    name=f"adsp{sc}")
                    for kk in range(SCK):
                        nc.vector.tensor_scalar(
                            out=ohs[:, kk, :], in0=iota_sb[:],
                            scalar1=edl[:, kk:kk + 1], scalar2=None,
                            op0=OP.is_equal)
                        tp = ppt.tile([P, P], bf, tag="tp",
                                      name=f"tp{sc}_{kk}")
                        nc.tensor.transpose(
                            out=tp[:], in_=ohs[:, kk, :],
                            identity=ident_sb[:])
                        nc.scalar.activation(ohT[:, kk, :], tp[:], AF.Copy)
                        # a_src(x_e) and one-hot-selected a_dst accumulate
                        # into the same PSUM region: es_raw = x_e.Ws1 + a_dst
                        for kt in range(KT):
                            nc.tensor.matmul(
                                out=adsp[:, kk, :],
                                lhsT=xg[:, kt, kk * P:(kk + 1) * P],
                                rhs=w1_sb[kt][:, HC:HC + H],
                                start=(kt == 0), stop=False)
                        nc.tensor.matmul(
                            out=adsp[:, kk, :], lhsT=ohT[:, kk, :],
                            rhs=adt_sb[:, win_of[sc][kk], 0:H],
                            start=False, stop=True)
                    es = pst.tile([P, SCK, H], f32, tag="es", name=f"es{sc}")
                    nc.vector.tensor_copy(out=es[:], in_=adsp[:, :, :])
                    elr = pst.tile([P, SCK, H], f32, tag="elr",
                                   name=f"elr{sc}")
                    nc.vector.scalar_tensor_tensor(
                        out=elr[:], in0=es[:], scalar=NEG_SLOPE, in1=es[:],
                        op0=OP.mult, op1=OP.max)
                    wts = pst.tile([P, SCK, H], bf, tag="wts", name=f"wts{sc}")
                    nc.scalar.activation(wts[:], elr[:], AF.Exp)
                    sc_cache[sc] = (xg, ohs, wts)
                    return sc_cache[sc]

                if mode == "ABG":
                    for sc in range(TSC):
                        get_sc(sc)
                for w in range(NWIN if mode in ("full", "NOCC") else 0):
                    pn1 = pps.tile([P, HC], f32, tag="pn1", name=f"pn1_{w}")
                    aux = pps.tile([P, 2 * H + AUG2], f32, tag="aux",
                                   name=f"aux{w}")
                    pd1 = aux[:, 0:H]
                    njw = len(window_chunks[w])
                    for i, (c, sc, kk) in enumerate(window_chunks[w]):
                        xg, ohs, wts = get_sc(sc)
                        he = phe.tile([P, HC], f32, tag="he",
                                      name=f"he{w}_{i}")
                        for kt in range(KT):
                            nc.tensor.matmul(
                                out=he[:],
                                lhsT=xg[:, kt, kk * P:(kk + 1) * P],
                                rhs=w1_sb[kt][:, 0:HC],
                                start=(kt == 0), stop=(kt == KT - 1))
                        hesb = pck.tile([P, HC], bf, tag="hesb",
                                        name=f"hesb{w}_{i}")
                        nc.scalar.activation(hesb[:], he[:], AF.Copy)
                        msg = pck.tile([P, HC], bf, tag="msg",
                                       name=f"msg{w}_{i}")
                        nc.vector.tensor_tensor(
                            out=msg[:].rearrange("p (c h) -> p c h", h=H),
                            in0=hesb[:].rearrange("p (c h) -> p c h", h=H),
                            in1=wts[:, kk:kk + 1, :].to_broadcast([P, C, H]),
                            op=OP.mult)
                        nc.tensor.matmul(
                            out=pn1[:], lhsT=ohs[:, kk, :], rhs=msg[:],
                            start=(i == 0), stop=False)
                        nc.tensor.matmul(
                            out=pd1, lhsT=ohs[:, kk, :], rhs=wts[:, kk, :],
                            start=(i == 0), stop=False)

                    # self-loop chunk: x rows of this window live in xT_sb,
                    # one-hot is the identity, a_dst read straight from adt_sb
                    hes = phe.tile([P, HC], f32, tag="he", name=f"hes{w}")
                    for kt in range(KT):
                        nc.tensor.matmul(
                            out=hes[:], lhsT=xT_sb[kt][:, w * P:(w + 1) * P],
                            rhs=w1_sb[kt][:, 0:HC],
                            start=(kt == 0), stop=(kt == KT - 1))
                    assl = psp.tile([P, H], f32, tag="assl", name=f"assl{w}")
                    for kt in range(KT):
                        nc.tensor.matmul(
                            out=assl[:],
                            lhsT=xT_sb[kt][:, w * P:(w + 1) * P],
                            rhs=w1_sb[kt][:, HC:HC + H],
                            start=(kt == 0), stop=False)
                    nc.tensor.matmul(
                        out=assl[:], lhsT=ident_sb[:],
                        rhs=adt_sb[:, w, 0:H], start=False, stop=True)
                    hesbs = pck.tile([P, HC], bf, tag="hesb",
                                     name=f"hesbs{w}")
                    nc.scalar.activation(hesbs[:], hes[:], AF.Copy)
                    ess = ppo.tile([P, H], f32, tag="ess", name=f"ess{w}")
                    nc.vector.tensor_copy(out=ess[:], in_=assl[:])
                    elrs = ppo.tile([P, H], f32, tag="elrs", name=f"elrs{w}")
                    nc.vector.scalar_tensor_tensor(
                        out=elrs[:], in0=ess[:], scalar=NEG_SLOPE,
                        in1=ess[:], op0=OP.mult, op1=OP.max)
                    wtss = ppo.tile([P, 1, H], bf, tag="wtss",
                                    name=f"wtss{w}")
                    nc.scalar.activation(wtss[:, 0, :], elrs[:], AF.Exp)
                    msgs = pck.tile([P, HC], bf, tag="msg", name=f"msgs{w}")
                    nc.vector.tensor_tensor(
                        out=msgs[:].rearrange("p (c h) -> p c h", h=H),
                        in0=hesbs[:].rearrange("p (c h) -> p c h", h=H),
                        in1=wtss[:, :, :].to_broadcast([P, C, H]),
                        op=OP.mult)
                    nc.tensor.matmul(
                        out=pn1[:], lhsT=ident_sb[:], rhs=msgs[:],
                        start=False, stop=True)
                    nc.tensor.matmul(
                        out=pd1, lhsT=ident_sb[:], rhs=wtss[:, 0, :],
                        start=False, stop=True)

                    den = ppo.tile([P, H], f32, tag="den", name=f"den{w}")
                    nc.vector.tensor_scalar(
                        out=den[:], in0=pd1, scalar1=1e-30, scalar2=None,
                        op0=OP.add)
                    rden = ppo.tile([P, H], f32, tag="rden", name=f"rden{w}")
                    nc.vector.reciprocal(out=rden[:], in_=den[:])
                    h1a = ppo.tile([P, HC], bf, tag="h1a", name=f"h1a{w}")
                    h1v = h1a[:].rearrange("p (c h) -> p c h", h=H)
                    pnv = pn1[:].rearrange("p (c h) -> p c h", h=H)
                    if not b1_nonzero:
                        for h in range(H):
                            nc.scalar.activation(
                                h1v[:, :, h:h + 1], pnv[:, :, h:h + 1],
                                AF.Relu, scale=rden[:, h:h + 1])
                    else:
                        t1 = ppo.tile([P, HC], f32, tag="t1", name=f"t1_{w}")
                        t1v = t1[:].rearrange("p (c h) -> p c h", h=H)
                        for h in range(H):
                            nc.scalar.activation(
                                t1v[:, :, h:h + 1], pnv[:, :, h:h + 1],
                                AF.Copy, scale=rden[:, h:h + 1])
                        nc.vector.tensor_tensor(
                            out=t1[:], in0=t1[:], in1=b1_sb[:], op=OP.add)
                        nc.vector.tensor_scalar(
                            out=h1a[:], in0=t1[:], scalar1=0.0, scalar2=None,
                            op0=OP.max)

                    # layer-2 pre-pass for this node tile
                    ph2 = aux[:, 2 * H:2 * H + AUG2]
                    for b in range(BT):
                        r = min(P, HC - b * P)
                        tp = ppt.tile([P, P], bf, tag="tp2w",
                                      name=f"tpw{w}_{b}")
                        nc.tensor.transpose(
                            out=tp[:r, :], in_=h1a[:, b * P:b * P + r],
                            identity=ident_sb[:])
                        h1T = ppo.tile([P, P], bf, tag="h1T",
                                       name=f"h1T{w}_{b}")
                        nc.scalar.activation(h1T[:r, :], tp[:r, :], AF.Copy)
                        nc.tensor.matmul(
                            out=ph2, lhsT=h1T[:r, :], rhs=w2_sb[b][:],
                            start=(b == 0), stop=(b == BT - 1))
                    nc.scalar.activation(hx2sb[:, w, :], ph2, AF.Copy)
                    nc.vector.tensor_copy(
                        out=adt_sb[:, w, H:H + 1],
                        in_=aux[:, 2 * H + OUT + 1:2 * H + OUT + 2])
                    r = rows_of(w)
                    nc.sync.dma_start(
                        out=hx2_dram[w * P:w * P + r, 0:AUG2],
                        in_=hx2sb[:r, w, :])

            # ---------------- AllGather layer-2 features
            if mode == "NOCC":
                nc.sync.dma_start(out=hx2_full[0:NSH, :],
                                  in_=hx2_dram[0:NSH, :])
            elif mode != "A":
                nc.gpsimd.collective_compute(
                    "AllGather", OP.bypass,
                    replica_groups=[list(range(cfg.NC))],
                    ins=[hx2_dram.opt()], outs=[hx2_full.opt()],
                )

            # ---------------- phase D: layer-2 edge loop + log_softmax
            with tc.tile_pool(name="phD_st", bufs=2) as pst, \
                 tc.tile_pool(name="phD_ck", bufs=4) as pck, \
                 tc.tile_pool(name="phD_po", bufs=2) as ppo, \
                 tc.tile_pool(name="phD_ps", bufs=2, space="PSUM") as pps, \
                 tc.tile_pool(name="phD_sp", bufs=1, space="PSUM") as psp, \
                 tc.tile_pool(name="phD_pt", bufs=2, space="PSUM") as ppt:

                sc2_cache = {}

                def get_sc2(sc):
                    if sc in sc2_cache:
                        return sc2_cache[sc]
                    six = pst.tile([P, SCK * 8], i16, tag="six2",
                                   name=f"s2ix{sc}")
                    nc.sync.dma_start(out=six[:], in_=eidx_t[sc, :, :])
                    edl = pst.tile([P, SCK], f32, tag="edl2",
                                   name=f"edl2_{sc}")
                    nc.sync.dma_start(out=edl[:], in_=edl_t[sc, :, :])
                    hxg = pst.tile([P, SCK, ADW], bf, tag="hxg2",
                                   name=f"hxg2_{sc}")
                    nc.gpsimd.dma_gather(
                        hxg[:], hx2_src(sc), six[:],
                        NIDX, NIDX, ADW, single_packet=False)
                    ohs = pst.tile([P, SCK, P], bf, tag="ohs2",
                                   name=f"ohs2_{sc}")
                    ohT = pst.tile([P, SCK, P], bf, tag="ohT2",
                                   name=f"ohT2_{sc}")
                    ade = psp.tile([P, SCK, 1], f32, tag="ade2",
                                   name=f"ade2_{sc}")
                    for kk in range(SCK):
                        nc.vector.tensor_scalar(
                            out=ohs[:, kk, :], in0=iota_sb[:],
                            scalar1=edl[:, kk:kk + 1], scalar2=None,
                            op0=OP.is_equal)
                        tp = ppt.tile([P, P], bf, tag="tp2",
                                      name=f"tp2_{sc}_{kk}")
                        nc.tensor.transpose(
                            out=tp[:], in_=ohs[:, kk, :],
                            identity=ident_sb[:])
                        nc.scalar.activation(ohT[:, kk, :], tp[:], AF.Copy)
                        nc.tensor.matmul(
                            out=ade[:, kk, :], lhsT=ohT[:, kk, :],
                            rhs=adt_sb[:, win_of[sc][kk], H:H + 1],
                            start=True, stop=True)
                    adesb = pst.tile([P, SCK, 1], f32, tag="adesb2",
                                     name=f"adesb2_{sc}")
                    nc.vector.tensor_copy(out=adesb[:], in_=ade[:, :, :])
                    es = pst.tile([P, SCK, 1], f32, tag="es2",
                                  name=f"es2_{sc}")
                    nc.vector.tensor_tensor(
                        out=es[:], in0=hxg[:, :, OUT:OUT + 1],
                        in1=adesb[:], op=OP.add)
                    elr = pst.tile([P, SCK, 1], f32, tag="elr2",
                                   name=f"elr2_{sc}")
                    nc.vector.scalar_tensor_tensor(
                        out=elr[:], in0=es[:], scalar=NEG_SLOPE, in1=es[:],
                        op0=OP.mult, op1=OP.max)
                    wts = pst.tile([P, SCK, 1], f32, tag="wts2",
                                   name=f"wts2_{sc}")
                    nc.scalar.activation(wts[:], elr[:], AF.Exp)
                    wtsb = pst.tile([P, SCK, 1], bf, tag="wtsb2",
                                    name=f"wtsb2_{sc}")
                    nc.vector.tensor_copy(out=wtsb[:], in_=wts[:])
                    sc2_cache[sc] = (hxg, ohs, wts, wtsb)
                    return sc2_cache[sc]

                if mode == "ABG":
                    for sc in range(TSC):
                        get_sc2(sc)
                if mode in ("A", "AB", "ABG"):
                    zt = ppo.tile([P, OUT], f32, tag="zt", name="zt")
                    nc.vector.memset(zt[:], 0.0)
                    for w in range(NWIN):
                        r = rows_of(w)
                        nc.sync.dma_start(
                            out=out_t[w * P:w * P + r, :], in_=zt[:r, :])
                for w in range(NWIN if mode not in ("A", "AB", "ABG") else 0):
                    pn2 = pps.tile([P, OUT], f32, tag="pn2", name=f"pn2_{w}")
                    pd2 = pps.tile([P, 1], f32, tag="pd2", name=f"pd2_{w}")
                    njw = len(window_chunks[w])
                    for i, (c, sc, kk) in enumerate(window_chunks[w]):
                        hxg, ohs, wts, wtsb = get_sc2(sc)
                        msg = pck.tile([P, OUT], bf, tag="msg2",
                                       name=f"ms2{w}_{i}")
                        nc.vector.tensor_scalar(
                            out=msg[:], in0=hxg[:, kk, 0:OUT],
                            scalar1=wts[:, kk, :], scalar2=None, op0=OP.mult)
                        nc.tensor.matmul(
                            out=pn2[:], lhsT=ohs[:, kk, :], rhs=msg[:],
                            start=(i == 0), stop=False)
                        nc.tensor.matmul(
                            out=pd2[:], lhsT=ohs[:, kk, :], rhs=wtsb[:, kk, :],
                            start=(i == 0), stop=False)

                    # layer-2 self-loop chunk from the SBUF-resident own rows
                    es2s = ppo.tile([P, 1], f32, tag="es2s", name=f"es2s{w}")
                    nc.vector.tensor_tensor(
                        out=es2s[:], in0=hx2sb[:, w, OUT:OUT + 1],
                        in1=adt_sb[:, w, H:H + 1], op=OP.add)
                    elr2s = ppo.tile([P, 1], f32, tag="elr2s",
                                     name=f"elr2s{w}")
                    nc.vector.scalar_tensor_tensor(
                        out=elr2s[:], in0=es2s[:], scalar=NEG_SLOPE,
                        in1=es2s[:], op0=OP.mult, op1=OP.max)
                    wt2s = ppo.tile([P, 1], f32, tag="wt2s", name=f"wt2s{w}")
                    nc.scalar.activation(wt2s[:], elr2s[:], AF.Exp)
                    wt2sb = ppo.tile([P, 1], bf, tag="wt2sb",
                                     name=f"wt2sb{w}")
                    nc.vector.tensor_copy(out=wt2sb[:], in_=wt2s[:])
                    msg2s = pck.tile([P, OUT], bf, tag="msg2",
                                     name=f"msg2s{w}")
                    nc.vector.tensor_scalar(
                        out=msg2s[:], in0=hx2sb[:, w, 0:OUT],
                        scalar1=wt2s[:, :], scalar2=None, op0=OP.mult)
                    nc.tensor.matmul(
                        out=pn2[:], lhsT=ident_sb[:], rhs=msg2s[:],
                        start=False, stop=True)
                    nc.tensor.matmul(
                        out=pd2[:], lhsT=ident_sb[:], rhs=wt2sb[:],
                        start=False, stop=True)

                    den = ppo.tile([P, 1], f32, tag="den2", name=f"den2_{w}")
                    nc.vector.tensor_scalar(
                        out=den[:], in0=pd2[:], scalar1=1e-30, scalar2=None,
                        op0=OP.add)
                    rden = ppo.tile([P, 1], f32, tag="rden2", name=f"rd2_{w}")
                    nc.vector.reciprocal(out=rden[:], in_=den[:])
                    o2 = ppo.tile([P, OUT], f32, tag="o2", name=f"o2_{w}")
                    nc.scalar.activation(
                        o2[:], pn2[:, 0:OUT], AF.Copy, scale=rden[:, 0:1])
                    if b2_nonzero:
                        nc.vector.tensor_tensor(
                            out=o2[:], in0=o2[:], in1=b2_sb[:], op=OP.add)
                    mx = ppo.tile([P, 1], f32, tag="mx", name=f"mx{w}")
                    nc.vector.reduce_max(
                        out=mx[:], in_=o2[:], axis=mybir.AxisListType.X)
                    negm = ppo.tile([P, 1], f32, tag="negm", name=f"negm{w}")
                    nc.vector.tensor_scalar(
                        out=negm[:], in0=mx[:], scalar1=-1.0, scalar2=None,
                        op0=OP.mult)
                    ex = ppo.tile([P, OUT], f32, tag="ex", name=f"ex{w}")
                    ssum = ppo.tile([P, 1], f32, tag="ssum", name=f"ssum{w}")
                    nc.scalar.activation(
                        ex[:], o2[:], AF.Exp, bias=negm[:, 0:1],
                        accum_out=ssum[:, 0:1])
                    lns = ppo.tile([P, 1], f32, tag="lns", name=f"lns{w}")
                    nc.scalar.activation(lns[:], ssum[:], AF.Ln)
                    sh = ppo.tile([P, 1], f32, tag="sh", name=f"sh{w}")
                    nc.vector.tensor_tensor(
                        out=sh[:], in0=negm[:], in1=lns[:], op=OP.subtract)
                    outt = ppo.tile([P, OUT], f32, tag="outt", name=f"outt{w}")
                    nc.scalar.activation(
                        outt[:], o2[:], AF.Identity, bias=sh[:, 0:1])
                    r = rows_of(w)
                    nc.sync.dma_start(
                        out=out_t[w * P:w * P + r, :], in_=outt[:r, :])

        dram_pool.__exit__(None, None, None)

    nc.compile()
    return nc


# ------------------------------------------------------------------ driver


def make_in_maps(cfg: Cfg, inputs: dict):
    x = np.asarray(inputs["x"], np.float32)
    edge_index = np.asarray(inputs["edge_index"])
    W1 = np.asarray(inputs["W1"], np.float32)
    a_src1 = np.asarray(inputs["a_src1"], np.float32)
    a_dst1 = np.asarray(inputs["a_dst1"], np.float32)
    b1 = np.asarray(inputs["b1"], np.float32)
    W2 = np.asarray(inputs["W2"], np.float32)
    a_src2 = np.asarray(inputs["a_src2"], np.float32)
    a_dst2 = np.asarray(inputs["a_dst2"], np.float32)
    b2 = np.asarray(inputs["b2"], np.float32)

    nch, TSC_c, eidx, edl = _schedule(cfg, edge_index)
    W1aug, W2aug = _prep_weights(cfg, W1, a_src1, a_dst1, W2, a_src2, a_dst2)
    iota = np.tile(np.arange(P, dtype=BF16), (P, 1))
    ident = np.eye(P, dtype=BF16)
    b1_nonzero = bool(np.any(b1))
    b2_nonzero = bool(np.any(b2))
    perm = _perm(cfg)

    NT = cfg.NWIN
    XPAD = NT * P
    xtab = x.astype(BF16)
    in_maps = []
    for k in range(cfg.NC):
        xs = x[k * cfg.NSH:(k + 1) * cfg.NSH]
        xTp = np.zeros((cfg.F, XPAD), BF16)
        xTp[:, :cfg.NSH] = xs.T.astype(BF16)
        m = {
            "xT": xTp,
            "xtab": xtab,
            "W1aug": W1aug,
            "W2aug": W2aug,
            "iota": iota,
            "ident": ident,
            "eidx": eidx[k],
            "edl": edl[k],
        }
        if b1_nonzero:
            m["b1rep"] = np.tile(b1[perm][None, :], (P, 1)).astype(np.float32)
        if b2_nonzero:
            m["b2rep"] = np.tile(b2[None, :], (P, 1)).astype(np.float32)
        in_maps.append(m)
    return in_maps, nch, TSC_c, b1_nonzero, b2_nonzero


class Executor:
    """Compile once; execute repeatedly through one jitted shard_map."""

    def __init__(self, cfg: Cfg, nch, TSC_c, b1nz, b2nz, mode="full"):
        import jax
        from jax.sharding import Mesh, PartitionSpec
        from jax.experimental.shard_map import shard_map
        from concourse import bass2jax
        import concourse.mybir as mybir_

        self.cfg = cfg
        nc = _build(cfg, nch, TSC_c, b1nz, b2nz, mode=mode)
        self.nc = nc
        bass2jax.install_neuronx_cc_hook()

        in_names, out_names, out_avals, zero_shapes = [], [], [], []
        for alloc in nc.m.functions[0].allocations:
            if not isinstance(alloc, mybir_.MemoryLocationSet):
                continue
            name = alloc.memorylocations[0].name
            if alloc.kind == "ExternalInput":
                in_names.append(name)
            elif alloc.kind == "ExternalOutput":
                shape = tuple(alloc.tensor_shape)
                dtype = mybir_.dt.np(alloc.dtype)
                out_avals.append(jax.core.ShapedArray(shape, dtype))
                out_names.append(name)
                zero_shapes.append((shape, dtype))
        assert nc.dbg_addr is None
        part_name = (nc.partition_id_tensor.name
                     if nc.partition_id_tensor else None)
        in_names = [n for n in in_names if n != part_name]
        n_params = len(in_names)
        all_names = in_names + out_names
        if part_name is not None:
            all_names = all_names + [part_name]
        donate = tuple(range(n_params, n_params + len(out_names)))

        def _body(*args):
            operands = list(args)
            if part_name is not None:
                operands.append(bass2jax.partition_id_tensor())
            outs = bass2jax._bass_exec_p.bind(
                *operands,
                out_avals=tuple(out_avals),
                in_names=tuple(all_names),
                out_names=tuple(out_names),
                lowering_input_output_aliases=(),
                sim_require_finite=True,
                sim_require_nnan=True,
                nc=nc,
            )
            return tuple(outs)

        devices = jax.devices()[:cfg.NC]
        mesh = Mesh(np.asarray(devices), ("core",))
        nio = n_params + len(out_names)
        self._fn = jax.jit(
            shard_map(
                _body, mesh=mesh,
                in_specs=(PartitionSpec("core"),) * nio,
                out_specs=(PartitionSpec("core"),) * len(out_names),
                check_rep=False,
            ),
            donate_argnums=donate, keep_unused=True,
        )
        self.in_names = in_names
        self.out_names = out_names
        self.out_avals = out_avals
        self.zero_shapes = zero_shapes
        self.mesh = mesh

    def bench(self, in_maps, iters=10):
        """Device-resident repeat timing: inputs uploaded once, outputs
        chained through donation. Returns per-iteration wall seconds."""
        import time
        import jax
        from jax.sharding import NamedSharding, PartitionSpec
        cfg = self.cfg
        sh = NamedSharding(self.mesh, PartitionSpec("core"))
        din = [
            jax.device_put(
                np.concatenate([np.asarray(m[n]) for m in in_maps], axis=0),
                sh)
            for n in self.in_names
        ]
        prev = [
            jax.device_put(np.zeros((cfg.NC * s[0], *s[1:]), d), sh)
            for s, d in self.zero_shapes
        ]
        outs = self._fn(*din, *prev)  # warm
        jax.block_until_ready(outs)
        times = []
        for _ in range(iters):
            t0 = time.perf_counter()
            outs = self._fn(*din, *outs)
            jax.block_until_ready(outs)
            times.append(time.perf_counter() - t0)
        return times

    def __call__(self, in_maps):
        cfg = self.cfg
        concat_in = [
            np.concatenate([np.asarray(m[n]) for m in in_maps], axis=0)
            for n in self.in_names
        ]
        concat_zeros = [
            np.zeros((cfg.NC * s[0], *s[1:]), d) for s, d in self.zero_shapes
        ]
        outs = self._fn(*concat_in, *concat_zeros)
        outs = [np.asarray(o) for o in outs]
        return [
            {
                n: outs[i].reshape(cfg.NC, *self.out_avals[i].shape)[c]
                for i, n in enumerate(self.out_names)
            }
            for c in range(cfg.NC)
        ]


_exec_cache = {}


def get_executor(cfg: Cfg, inputs: dict, mode="full"):
    in_maps, nch, TSC_c, b1nz, b2nz = make_in_maps(cfg, inputs)
    key = (cfg, tuple(tuple(c) for c in nch), b1nz, b2nz, mode)
    if key not in _exec_cache:
        _exec_cache[key] = Executor(cfg, nch, TSC_c, b1nz, b2nz, mode=mode)
    return _exec_cache[key], in_maps


def run(cfg: Cfg, inputs: dict, trace: bool = False):
    ex, in_maps = get_executor(cfg, inputs)
    results = ex(in_maps)
    out = np.concatenate([results[k]["out"] for k in range(cfg.NC)], 0)
    return out, ex


def kernel(**inputs) -> np.ndarray:
    out, _ = run(FULL, inputs)
    return out.astype(np.float32)


# revision 37
# speedup vs baseline: 1.0013x; 1.0013x over previous
"""GAT (2-layer graph attention network) on 8 Trainium2 NeuronCores.

Strategy (dst-sharded graph parallel, gather-x-recompute):
  - Nodes are partitioned across the 8 cores (6250 dst nodes each).
  - Host pre-sorts edges (incl. self-loops) by destination, groups them into
    128-dst "windows" and 128-edge "chunks", padding so every core runs an
    identical static program.  Chunks are segregated by source half (int16
    gather-index range), and batched into 2048-edge superchunks.
  - Layer 1 avoids any feature AllGather: each core holds the FULL x as a
    gather table; per superchunk one transposed dma_gather fetches raw x rows
    (512B each) feature-major, and h|alpha_src are recomputed PER EDGE on the
    tensor engine (x_edge^T @ W1aug) - the PE is otherwise idle and this cuts
    gather descriptors/bytes ~3x vs gathering precomputed h rows.
  - alpha_dst needs no gather at all: a_dst lives in an SBUF table indexed by
    (window, slot); the per-chunk one-hot is transposed on the PE and a tiny
    matmul selects per-edge a_dst.
  - Scatter-add into PSUM via one-hot matmuls per 128-dst window; softmax
    denominators from a second matmul against the same one-hot.
  - Layer 2: h2 rows are dst-core-local, so a small AllGather ([N,128] bf16)
    replicates them; per-edge rows gathered as before; a_dst2 via the same
    one-hot-transpose trick; then log_softmax.
"""

import math
from dataclasses import dataclass

import ml_dtypes
import numpy as np

import concourse.bass as bass
import concourse.mybir as mybir
import concourse.tile as tile
from concourse import bacc

BF16 = ml_dtypes.bfloat16
P = 128
NEG_SLOPE = 0.2
HALF = 25000  # src-table split point (int16 index range)


@dataclass(frozen=True)
class Cfg:
    N: int = 50000
    F: int = 256
    H: int = 8
    C: int = 64
    OUT: int = 40
    NC: int = 8
    SCK: int = 32          # chunks per gather superchunk

    @property
    def HC(self):
        return self.H * self.C

    @property
    def NSH(self):
        return self.N // self.NC

    @property
    def NWIN(self):
        return math.ceil(self.NSH / P)

    @property
    def nclass(self):
        return 2 if self.N > 32767 else 1


FULL = Cfg()


# ---------------------------------------------------------------- host side


def _schedule(cfg: Cfg, edge_index: np.ndarray):
    """Sort/pad edges into the common static chunk schedule.

    Returns (nch, TSC_c, eidx, edl):
      nch  : [nclass][NWIN] chunks per (class, window), common to all cores
      eidx : int16 [NC, TSC, 128, SCK*8]  src local-to-table indices
             (idx i of a superchunk at [i % 16 (+16g replicas), i // 16])
      edl  : float32 [NC, TSC, 128, SCK] dst-in-window (pad -1)
    """
    N, NCOR, NSH, SCK = cfg.N, cfg.NC, cfg.NSH, cfg.SCK
    NWIN, NCLS = cfg.NWIN, cfg.nclass
    # self-loops are handled on-chip per window (identity one-hot, local x),
    # so only the real edges enter the gathered chunk stream
    src = np.asarray(edge_index[0]).astype(np.int64)
    dst = np.asarray(edge_index[1]).astype(np.int64)
    core = (dst // NSH).astype(np.int32)
    dl = (dst % NSH).astype(np.int32)
    win = dl // P
    wloc = dl % P
    cls = (src >= HALF).astype(np.int32) if NCLS == 2 else np.zeros_like(core)

    counts = np.zeros((NCOR, NCLS, NWIN), np.int64)
    np.add.at(counts, (core, cls, win), 1)
    nch = np.maximum(np.ceil(counts / P).astype(np.int64).max(axis=0), 1)
    T_c = [int(nch[c].sum()) for c in range(NCLS)]
    TSC_c = [math.ceil(t / SCK) for t in T_c]
    TSC = sum(TSC_c)
    TP_c = [t * SCK for t in TSC_c]

    # slot offset of (class, window) inside its class stream (in edges)
    wstart = np.zeros((NCLS, NWIN + 1), np.int64)
    for c in range(NCLS):
        wstart[c, 1:] = np.cumsum(nch[c])
    wstart *= P

    eidx = np.zeros((NCOR, TSC, P, SCK * 8), np.int16)
    edlA = np.full((NCOR, TSC, P, SCK), -1.0, np.float32)
    sc_base = [0, TSC_c[0]] if NCLS == 2 else [0]

    for k in range(NCOR):
        for c in range(NCLS):
            m = (core == k) & (cls == c)
            s_k = src[m] - (HALF if c == 1 else 0)
            w_k = win[m]
            wl_k = wloc[m]
            order = np.lexsort((s_k, w_k))
            s_k, w_k, wl_k = s_k[order], w_k[order], wl_k[order]
            cnts = counts[k, c]
            offs = np.concatenate([[0], np.cumsum(cnts[:-1])])
            slot = wstart[c, w_k] + (np.arange(len(s_k)) - offs[w_k])
            es = np.zeros(TP_c[c] * P, np.int16)
            el = np.full(TP_c[c] * P, -1.0, np.float32)
            es[slot] = s_k.astype(np.int16)
            el[slot] = wl_k.astype(np.float32)
            # flat i -> idx tile [i%16, i//16] (replicated), edl [i%128, i//128]
            for si in range(TSC_c[c]):
                sc = sc_base[c] + si
                seg_s = es[si * SCK * P:(si + 1) * SCK * P]
                seg_l = el[si * SCK * P:(si + 1) * SCK * P]
                t16s = seg_s.reshape(-1, 16).T  # [16, SCK*8]
                eidx[k, sc, :, :] = np.tile(t16s, (8, 1))
                edlA[k, sc] = seg_l.reshape(SCK, P).T
    nch_py = [[int(x) for x in nch[c]] for c in range(NCLS)]
    return nch_py, TSC_c, eidx, edlA


def _perm(cfg: Cfg):
    p = np.empty(cfg.HC, np.int64)
    for h in range(cfg.H):
        p[np.arange(cfg.C) * cfg.H + h] = h * cfg.C + np.arange(cfg.C)
    return p


def _prep_weights(cfg: Cfg, W1, a_src1, a_dst1, W2, a_src2, a_dst2):
    perm = _perm(cfg)
    H, C, HC, OUT = cfg.H, cfg.C, cfg.HC, cfg.OUT
    Ws1 = np.stack([W1[:, h * C:(h + 1) * C] @ a_src1[h] for h in range(H)], 1)
    Wd1 = np.stack([W1[:, h * C:(h + 1) * C] @ a_dst1[h] for h in range(H)], 1)
    W1aug = np.concatenate([W1[:, perm], Ws1, Wd1], axis=1).astype(BF16)
    w2s = (W2 @ a_src2[0])[:, None]
    w2d = (W2 @ a_dst2[0])[:, None]
    L2PAD = 48 - (OUT + 2)
    W2aug = np.concatenate(
        [W2, w2s, w2d, np.zeros((HC, L2PAD), W2.dtype)], axis=1
    )[perm, :].astype(BF16)
    return W1aug, W2aug


# -------------------------------------------------------------- device side


def _build(cfg: Cfg, nch, TSC_c, b1_nonzero=False, b2_nonzero=False,
           mode="full"):
    N, F, H, C, HC, OUT = cfg.N, cfg.F, cfg.H, cfg.C, cfg.HC, cfg.OUT
    NSH, SCK, NWIN = cfg.NSH, cfg.SCK, cfg.NWIN
    NCLS = cfg.nclass
    TSC = sum(TSC_c)
    KT = math.ceil(F / P)
    BT = math.ceil(HC / P)
    AUG1 = HC + 2 * H
    AUG2 = 48
    ADW = 128                    # hx2 table row stride (elements)
    NT = NWIN
    XPAD = NT * P
    NIDX = SCK * P

    bf = mybir.dt.bfloat16
    f32 = mybir.dt.float32
    i16 = mybir.dt.int16
    AF = mybir.ActivationFunctionType
    OP = mybir.AluOpType

    nc = bacc.Bacc(
        "TRN2", target_bir_lowering=False, debug=False,
        enable_asserts=False, num_devices=cfg.NC,
    )

    xT_t = nc.dram_tensor("xT", [F, XPAD], bf, kind="ExternalInput")
    xtab_t = nc.dram_tensor("xtab", [N, F], bf, kind="ExternalInput")
    w1_t = nc.dram_tensor("W1aug", [F, AUG1], bf, kind="ExternalInput")
    w2_t = nc.dram_tensor("W2aug", [HC, AUG2], bf, kind="ExternalInput")
    iota_t = nc.dram_tensor("iota", [P, P], bf, kind="ExternalInput")
    ident_t = nc.dram_tensor("ident", [P, P], bf, kind="ExternalInput")
    eidx_t = nc.dram_tensor("eidx", [TSC, P, SCK * 8], i16,
                            kind="ExternalInput")
    edl_t = nc.dram_tensor("edl", [TSC, P, SCK], f32, kind="ExternalInput")
    if b1_nonzero:
        b1_t = nc.dram_tensor("b1rep", [P, HC], f32, kind="ExternalInput")
    if b2_nonzero:
        b2_t = nc.dram_tensor("b2rep", [P, OUT], f32, kind="ExternalInput")
    out_t = nc.dram_tensor("out", [NSH, OUT], f32, kind="ExternalOutput")

    # (class, sc, kk) schedule per window: class streams are contiguous
    sc_base = [0, TSC_c[0]] if NCLS == 2 else [0]
    window_chunks = []  # per window: list of (c, sc, kk)
    pos_c = [0] * NCLS
    for w in range(NWIN):
        lst = []
        for c in range(NCLS):
            for _ in range(nch[c][w]):
                j = pos_c[c]
                lst.append((c, sc_base[c] + j // SCK, j % SCK))
                pos_c[c] += 1
        window_chunks.append(lst)
    rows_of = lambda w: min(P, NSH - w * P)
    # class of each sc (for table selection) and window of each (sc, kk)
    sc_cls = [0] * TSC
    if NCLS == 2:
        for s in range(TSC_c[0], TSC):
            sc_cls[s] = 1
    win_of = [[0] * SCK for _ in range(TSC)]
    for w in range(NWIN):
        for (c, sc, kk) in window_chunks[w]:
            win_of[sc][kk] = w

    with tile.TileContext(nc) as tc:
        dram_pool = tc.tile_pool(name="dram", bufs=1, space="DRAM")
        pdr = dram_pool.__enter__()
        shared_as = "Shared" if cfg.NC > 4 else "Local"
        hx2_dram = pdr.tile([NSH, ADW], bf, name="hx2_dram")
        hx2_full = pdr.tile([N, ADW], bf, addr_space=shared_as,
                            name="hx2_full")

        def xtab_src(sc):
            if NCLS == 1 or sc_cls[sc] == 0:
                return xtab_t[0:min(HALF, N), :]
            return xtab_t[HALF:N, :]

        def hx2_src(sc):
            if NCLS == 1 or sc_cls[sc] == 0:
                return hx2_full[0:min(HALF, N), :]
            return hx2_full[HALF:N, :]

        with tc.tile_pool(name="const", bufs=1) as pc:
            iota_sb = pc.tile([P, P], bf, name="iota_sb")
            nc.sync.dma_start(out=iota_sb[:], in_=iota_t[:, :])
            ident_sb = pc.tile([P, P], bf, name="ident_sb")
            nc.sync.dma_start(out=ident_sb[:], in_=ident_t[:, :])
            w1_sb = []
            for kk in range(KT):
                r = min(P, F - kk * P)
                t_ = pc.tile([r, AUG1], bf, name=f"w1_sb{kk}")
                nc.sync.dma_start(out=t_[:], in_=w1_t[kk * P:kk * P + r, :])
                w1_sb.append(t_)
            w2_sb = []
            for b in range(BT):
                r = min(P, HC - b * P)
                t_ = pc.tile([r, AUG2], bf, name=f"w2_sb{b}")
                nc.sync.dma_start(out=t_[:], in_=w2_t[b * P:b * P + r, :])
                w2_sb.append(t_)
            xT_sb = []
            for kk in range(KT):
                r = min(P, F - kk * P)
                t_ = pc.tile([r, XPAD], bf, name=f"xT_sb{kk}")
                nc.sync.dma_start(out=t_[:], in_=xT_t[kk * P:kk * P + r, :])
                xT_sb.append(t_)
            if b1_nonzero:
                b1_sb = pc.tile([P, HC], f32, name="b1_sb")
                nc.sync.dma_start(out=b1_sb[:], in_=b1_t[:, :])
            if b2_nonzero:
                b2_sb = pc.tile([P, OUT], f32, name="b2_sb")
                nc.sync.dma_start(out=b2_sb[:], in_=b2_t[:, :])
            # SBUF a_dst table: [slot, window, head(8) + l2(1)]
            adt_sb = pc.tile([P, NWIN, H + 1], bf, name="adt_sb")
            # SBUF copy of this core's own hx2 rows (for layer-2 self-loops)
            hx2sb = pc.tile([P, NWIN, AUG2], bf, name="hx2sb")

            # ---------------- phase A: a_dst table = (x @ Wd1) per window
            with tc.tile_pool(name="phA_ps", bufs=2, space="PSUM") as pap:
                for t in range(NT):
                    pa = pap.tile([P, 2 * H], f32, tag="pA", name=f"pA{t}")
                    for kk in range(KT):
                        lhsT = xT_sb[kk][:, t * P:(t + 1) * P]
                        nc.tensor.matmul(
                            out=pa[:], lhsT=lhsT, rhs=w1_sb[kk][:, HC:AUG1],
                            start=(kk == 0), stop=(kk == KT - 1))
                    nc.vector.tensor_copy(
                        out=adt_sb[:, t, 0:H], in_=pa[:, H:2 * H])

            # ---------------- phase C: layer-1 edge loop (+ h2 per window)
            with tc.tile_pool(name="phC_st", bufs=2) as pst, \
                 tc.tile_pool(name="phC_ck", bufs=4) as pck, \
                 tc.tile_pool(name="phC_po", bufs=2) as ppo, \
                 tc.tile_pool(name="phC_he", bufs=1, space="PSUM") as phe, \
                 tc.tile_pool(name="phC_ps", bufs=1, space="PSUM") as pps, \
                 tc.tile_pool(name="phC_sp", bufs=1, space="PSUM") as psp, \
                 tc.tile_pool(name="phC_pt", bufs=1, space="PSUM") as ppt:

                sc_cache = {}

                def get_sc(sc):
                    if sc in sc_cache:
                        return sc_cache[sc]
                    six = pst.tile([P, SCK * 8], i16, tag="six",
                                   name=f"six{sc}")
                    nc.sync.dma_start(out=six[:], in_=eidx_t[sc, :, :])
                    edl = pst.tile([P, SCK], f32, tag="edl", name=f"edl{sc}")
                    nc.sync.dma_start(out=edl[:], in_=edl_t[sc, :, :])
                    xg = pst.tile([P, KT, NIDX], bf, tag="xg", name=f"xg{sc}")
                    nc.gpsimd.dma_gather(
                        xg[:], xtab_src(sc), six[:],
                        NIDX, NIDX, F, transpose=True, single_packet=False)
                    ohs = pst.tile([P, SCK, P], bf, tag="ohs", name=f"ohs{sc}")
                    ohT = pst.tile([P, SCK, P], bf, tag="ohT", name=f"ohT{sc}")
                    adsp = psp.tile([P, SCK, H], f32, tag="adsp",
                                    name=f"adsp{sc}")
                    for kk in range(SCK):
                        nc.vector.tensor_scalar(
                            out=ohs[:, kk, :], in0=iota_sb[:],
                            scalar1=edl[:, kk:kk + 1], scalar2=None,
                            op0=OP.is_equal)
                        tp = ppt.tile([P, P], bf, tag="tp",
                                      name=f"tp{sc}_{kk}")
                        nc.tensor.transpose(
                            out=tp[:], in_=ohs[:, kk, :],
                            identity=ident_sb[:])
                        nc.scalar.activation(ohT[:, kk, :], tp[:], AF.Copy)
                        # a_src(x_e) and one-hot-selected a_dst accumulate
                        # into the same PSUM region: es_raw = x_e.Ws1 + a_dst
                        for kt in range(KT):
                            nc.tensor.matmul(
                                out=adsp[:, kk, :],
                                lhsT=xg[:, kt, kk * P:(kk + 1) * P],
                                rhs=w1_sb[kt][:, HC:HC + H],
                                start=(kt == 0), stop=False)
                        nc.tensor.matmul(
                            out=adsp[:, kk, :], lhsT=ohT[:, kk, :],
                            rhs=adt_sb[:, win_of[sc][kk], 0:H],
                            start=False, stop=True)
                    es = pst.tile([P, SCK, H], f32, tag="es", name=f"es{sc}")
                    nc.vector.tensor_copy(out=es[:], in_=adsp[:, :, :])
                    elr = pst.tile([P, SCK, H], f32, tag="elr",
                                   name=f"elr{sc}")
                    nc.vector.scalar_tensor_tensor(
                        out=elr[:], in0=es[:], scalar=NEG_SLOPE, in1=es[:],
                        op0=OP.mult, op1=OP.max)
                    wts = pst.tile([P, SCK, H], bf, tag="wts", name=f"wts{sc}")
                    nc.scalar.activation(wts[:], elr[:], AF.Exp)
                    sc_cache[sc] = (xg, ohs, wts)
                    return sc_cache[sc]

                if mode == "ABG":
                    for sc in range(TSC):
                        get_sc(sc)
                for w in range(NWIN if mode in ("full", "NOCC") else 0):
                    pn1 = pps.tile([P, HC], f32, tag="pn1", name=f"pn1_{w}")
                    aux = pps.tile([P, 2 * H + AUG2], f32, tag="aux",
                                   name=f"aux{w}")
                    pd1 = aux[:, 0:H]
                    njw = len(window_chunks[w])
                    for i, (c, sc, kk) in enumerate(window_chunks[w]):
                        xg, ohs, wts = get_sc(sc)
                        he = phe.tile([P, HC], f32, tag="he",
                                      name=f"he{w}_{i}")
                        for kt in range(KT):
                            nc.tensor.matmul(
                                out=he[:],
                                lhsT=xg[:, kt, kk * P:(kk + 1) * P],
                                rhs=w1_sb[kt][:, 0:HC],
                                start=(kt == 0), stop=(kt == KT - 1))
                        hesb = pck.tile([P, HC], bf, tag="hesb",
                                        name=f"hesb{w}_{i}")
                        nc.scalar.activation(hesb[:], he[:], AF.Copy)
                        msg = pck.tile([P, HC], bf, tag="msg",
                                       name=f"msg{w}_{i}")
                        nc.vector.tensor_tensor(
                            out=msg[:].rearrange("p (c h) -> p c h", h=H),
                            in0=hesb[:].rearrange("p (c h) -> p c h", h=H),
                            in1=wts[:, kk:kk + 1, :].to_broadcast([P, C, H]),
                            op=OP.mult)
                        nc.tensor.matmul(
                            out=pn1[:], lhsT=ohs[:, kk, :], rhs=msg[:],
                            start=(i == 0), stop=False)
                        nc.tensor.matmul(
                            out=pd1, lhsT=ohs[:, kk, :], rhs=wts[:, kk, :],
                            start=(i == 0), stop=False)

                    # self-loop chunk: x rows of this window live in xT_sb,
                    # one-hot is the identity, a_dst read straight from adt_sb
                    hes = phe.tile([P, HC], f32, tag="he", name=f"hes{w}")
                    for kt in range(KT):
                        nc.tensor.matmul(
                            out=hes[:], lhsT=xT_sb[kt][:, w * P:(w + 1) * P],
                            rhs=w1_sb[kt][:, 0:HC],
                            start=(kt == 0), stop=(kt == KT - 1))
                    assl = psp.tile([P, H], f32, tag="assl", name=f"assl{w}")
                    for kt in range(KT):
                        nc.tensor.matmul(
                            out=assl[:],
                            lhsT=xT_sb[kt][:, w * P:(w + 1) * P],
                            rhs=w1_sb[kt][:, HC:HC + H],
                            start=(kt == 0), stop=False)
                    nc.tensor.matmul(
                        out=assl[:], lhsT=ident_sb[:],
                        rhs=adt_sb[:, w, 0:H], start=False, stop=True)
                    hesbs = pck.tile([P, HC], bf, tag="hesb",
                                     name=f"hesbs{w}")
                    nc.scalar.activation(hesbs[:], hes[:], AF.Copy)
                    ess = ppo.tile([P, H], f32, tag="ess", name=f"ess{w}")
                    nc.vector.tensor_copy(out=ess[:], in_=assl[:])
                    elrs = ppo.tile([P, H], f32, tag="elrs", name=f"elrs{w}")
                    nc.vector.scalar_tensor_tensor(
                        out=elrs[:], in0=ess[:], scalar=NEG_SLOPE,
                        in1=ess[:], op0=OP.mult, op1=OP.max)
                    wtss = ppo.tile([P, 1, H], bf, tag="wtss",
                                    name=f"wtss{w}")
                    nc.scalar.activation(wtss[:, 0, :], elrs[:], AF.Exp)
                    msgs = pck.tile([P, HC], bf, tag="msg", name=f"msgs{w}")
                    nc.vector.tensor_tensor(
                        out=msgs[:].rearrange("p (c h) -> p c h", h=H),
                        in0=hesbs[:].rearrange("p (c h) -> p c h", h=H),
                        in1=wtss[:, :, :].to_broadcast([P, C, H]),
                        op=OP.mult)
                    nc.tensor.matmul(
                        out=pn1[:], lhsT=ident_sb[:], rhs=msgs[:],
                        start=False, stop=True)
                    nc.tensor.matmul(
                        out=pd1, lhsT=ident_sb[:], rhs=wtss[:, 0, :],
                        start=False, stop=True)

                    den = ppo.tile([P, H], f32, tag="den", name=f"den{w}")
                    nc.vector.tensor_scalar(
                        out=den[:], in0=pd1, scalar1=1e-30, scalar2=None,
                        op0=OP.add)
                    rden = ppo.tile([P, H], f32, tag="rden", name=f"rden{w}")
                    nc.vector.reciprocal(out=rden[:], in_=den[:])
                    h1a = ppo.tile([P, HC], bf, tag="h1a", name=f"h1a{w}")
                    h1v = h1a[:].rearrange("p (c h) -> p c h", h=H)
                    pnv = pn1[:].rearrange("p (c h) -> p c h", h=H)
                    if not b1_nonzero:
                        for h in range(H):
                            nc.scalar.activation(
                                h1v[:, :, h:h + 1], pnv[:, :, h:h + 1],
                                AF.Relu, scale=rden[:, h:h + 1])
                    else:
                        t1 = ppo.tile([P, HC], f32, tag="t1", name=f"t1_{w}")
                        t1v = t1[:].rearrange("p (c h) -> p c h", h=H)
                        for h in range(H):
                            nc.scalar.activation(
                                t1v[:, :, h:h + 1], pnv[:, :, h:h + 1],
                                AF.Copy, scale=rden[:, h:h + 1])
                        nc.vector.tensor_tensor(
                            out=t1[:], in0=t1[:], in1=b1_sb[:], op=OP.add)
                        nc.vector.tensor_scalar(
                            out=h1a[:], in0=t1[:], scalar1=0.0, scalar2=None,
                            op0=OP.max)

                    # layer-2 pre-pass for this node tile
                    ph2 = aux[:, 2 * H:2 * H + AUG2]
                    for b in range(BT):
                        r = min(P, HC - b * P)
                        tp = ppt.tile([P, P], bf, tag="tp2w",
                                      name=f"tpw{w}_{b}")
                        nc.tensor.transpose(
                            out=tp[:r, :], in_=h1a[:, b * P:b * P + r],
                            identity=ident_sb[:])
                        h1T = ppo.tile([P, P], bf, tag="h1T",
                                       name=f"h1T{w}_{b}")
                        nc.scalar.activation(h1T[:r, :], tp[:r, :], AF.Copy)
                        nc.tensor.matmul(
                            out=ph2, lhsT=h1T[:r, :], rhs=w2_sb[b][:],
                            start=(b == 0), stop=(b == BT - 1))
                    nc.scalar.activation(hx2sb[:, w, :], ph2, AF.Copy)
                    nc.vector.tensor_copy(
                        out=adt_sb[:, w, H:H + 1],
                        in_=aux[:, 2 * H + OUT + 1:2 * H + OUT + 2])
                    r = rows_of(w)
                    nc.sync.dma_start(
                        out=hx2_dram[w * P:w * P + r, 0:AUG2],
                        in_=hx2sb[:r, w, :])

            # ---------------- AllGather layer-2 features
            if mode == "NOCC":
                nc.sync.dma_start(out=hx2_full[0:NSH, :],
                                  in_=hx2_dram[0:NSH, :])
            elif mode != "A":
                nc.gpsimd.collective_compute(
                    "AllGather", OP.bypass,
                    replica_groups=[list(range(cfg.NC))],
                    ins=[hx2_dram.opt()], outs=[hx2_full.opt()],
                )

            # ---------------- phase D: layer-2 edge loop + log_softmax
            with tc.tile_pool(name="phD_st", bufs=2) as pst, \
                 tc.tile_pool(name="phD_ck", bufs=4) as pck, \
                 tc.tile_pool(name="phD_po", bufs=2) as ppo, \
                 tc.tile_pool(name="phD_ps", bufs=2, space="PSUM") as pps, \
                 tc.tile_pool(name="phD_sp", bufs=1, space="PSUM") as psp, \
                 tc.tile_pool(name="phD_pt", bufs=2, space="PSUM") as ppt:

                sc2_cache = {}

                def get_sc2(sc):
                    if sc in sc2_cache:
                        return sc2_cache[sc]
                    six = pst.tile([P, SCK * 8], i16, tag="six2",
                                   name=f"s2ix{sc}")
                    nc.sync.dma_start(out=six[:], in_=eidx_t[sc, :, :])
                    edl = pst.tile([P, SCK], f32, tag="edl2",
                                   name=f"edl2_{sc}")
                    nc.sync.dma_start(out=edl[:], in_=edl_t[sc, :, :])
                    hxg = pst.tile([P, SCK, ADW], bf, tag="hxg2",
                                   name=f"hxg2_{sc}")
                    nc.gpsimd.dma_gather(
                        hxg[:], hx2_src(sc), six[:],
                        NIDX, NIDX, ADW, single_packet=False)
                    ohs = pst.tile([P, SCK, P], bf, tag="ohs2",
                                   name=f"ohs2_{sc}")
                    ohT = pst.tile([P, SCK, P], bf, tag="ohT2",
                                   name=f"ohT2_{sc}")
                    ade = psp.tile([P, SCK, 1], f32, tag="ade2",
                                   name=f"ade2_{sc}")
                    for kk in range(SCK):
                        nc.vector.tensor_scalar(
                            out=ohs[:, kk, :], in0=iota_sb[:],
                            scalar1=edl[:, kk:kk + 1], scalar2=None,
                            op0=OP.is_equal)
                        tp = ppt.tile([P, P], bf, tag="tp2",
                                      name=f"tp2_{sc}_{kk}")
                        nc.tensor.transpose(
                            out=tp[:], in_=ohs[:, kk, :],
                            identity=ident_sb[:])
                        nc.scalar.activation(ohT[:, kk, :], tp[:], AF.Copy)
                        nc.tensor.matmul(
                            out=ade[:, kk, :], lhsT=ohT[:, kk, :],
                            rhs=adt_sb[:, win_of[sc][kk], H:H + 1],
                            start=True, stop=True)
                    adesb = pst.tile([P, SCK, 1], f32, tag="adesb2",
                                     name=f"adesb2_{sc}")
                    nc.vector.tensor_copy(out=adesb[:], in_=ade[:, :, :])
                    es = pst.tile([P, SCK, 1], f32, tag="es2",
                                  name=f"es2_{sc}")
                    nc.vector.tensor_tensor(
                        out=es[:], in0=hxg[:, :, OUT:OUT + 1],
                        in1=adesb[:], op=OP.add)
                    elr = pst.tile([P, SCK, 1], f32, tag="elr2",
                                   name=f"elr2_{sc}")
                    nc.vector.scalar_tensor_tensor(
                        out=elr[:], in0=es[:], scalar=NEG_SLOPE, in1=es[:],
                        op0=OP.mult, op1=OP.max)
                    wts = pst.tile([P, SCK, 1], f32, tag="wts2",
                                   name=f"wts2_{sc}")
                    nc.scalar.activation(wts[:], elr[:], AF.Exp)
                    wtsb = pst.tile([P, SCK, 1], bf, tag="wtsb2",
                                    name=f"wtsb2_{sc}")
                    nc.vector.tensor_copy(out=wtsb[:], in_=wts[:])
                    sc2_cache[sc] = (hxg, ohs, wts, wtsb)
                    return sc2_cache[sc]

                if mode == "ABG":
                    for sc in range(TSC):
                        get_sc2(sc)
                if mode in ("A", "AB", "ABG"):
                    zt = ppo.tile([P, OUT], f32, tag="zt", name="zt")
                    nc.vector.memset(zt[:], 0.0)
                    for w in range(NWIN):
                        r = rows_of(w)
                        nc.sync.dma_start(
                            out=out_t[w * P:w * P + r, :], in_=zt[:r, :])
                for w in range(NWIN if mode not in ("A", "AB", "ABG") else 0):
                    pn2 = pps.tile([P, OUT], f32, tag="pn2", name=f"pn2_{w}")
                    pd2 = pps.tile([P, 1], f32, tag="pd2", name=f"pd2_{w}")
                    njw = len(window_chunks[w])
                    for i, (c, sc, kk) in enumerate(window_chunks[w]):
                        hxg, ohs, wts, wtsb = get_sc2(sc)
                        msg = pck.tile([P, OUT], bf, tag="msg2",
                                       name=f"ms2{w}_{i}")
                        nc.vector.tensor_scalar(
                            out=msg[:], in0=hxg[:, kk, 0:OUT],
                            scalar1=wts[:, kk, :], scalar2=None, op0=OP.mult)
                        nc.tensor.matmul(
                            out=pn2[:], lhsT=ohs[:, kk, :], rhs=msg[:],
                            start=(i == 0), stop=False)
                        nc.tensor.matmul(
                            out=pd2[:], lhsT=ohs[:, kk, :], rhs=wtsb[:, kk, :],
                            start=(i == 0), stop=False)

                    # layer-2 self-loop chunk from the SBUF-resident own rows
                    es2s = ppo.tile([P, 1], f32, tag="es2s", name=f"es2s{w}")
                    nc.vector.tensor_tensor(
                        out=es2s[:], in0=hx2sb[:, w, OUT:OUT + 1],
                        in1=adt_sb[:, w, H:H + 1], op=OP.add)
                    elr2s = ppo.tile([P, 1], f32, tag="elr2s",
                                     name=f"elr2s{w}")
                    nc.vector.scalar_tensor_tensor(
                        out=elr2s[:], in0=es2s[:], scalar=NEG_SLOPE,
                        in1=es2s[:], op0=OP.mult, op1=OP.max)
                    wt2s = ppo.tile([P, 1], f32, tag="wt2s", name=f"wt2s{w}")
                    nc.scalar.activation(wt2s[:], elr2s[:], AF.Exp)
                    wt2sb = ppo.tile([P, 1], bf, tag="wt2sb",
                                     name=f"wt2sb{w}")
                    nc.vector.tensor_copy(out=wt2sb[:], in_=wt2s[:])
                    msg2s = pck.tile([P, OUT], bf, tag="msg2",
                                     name=f"msg2s{w}")
                    nc.vector.tensor_scalar(
                        out=msg2s[:], in0=hx2sb[:, w, 0:OUT],
                        scalar1=wt2s[:, :], scalar2=None, op0=OP.mult)
                    nc.tensor.matmul(
                        out=pn2[:], lhsT=ident_sb[:], rhs=msg2s[:],
                        start=False, stop=True)
                    nc.tensor.matmul(
                        out=pd2[:], lhsT=ident_sb[:], rhs=wt2sb[:],
                        start=False, stop=True)

                    den = ppo.tile([P, 1], f32, tag="den2", name=f"den2_{w}")
                    nc.vector.tensor_scalar(
                        out=den[:], in0=pd2[:], scalar1=1e-30, scalar2=None,
                        op0=OP.add)
                    rden = ppo.tile([P, 1], f32, tag="rden2", name=f"rd2_{w}")
                    nc.vector.reciprocal(out=rden[:], in_=den[:])
                    o2 = ppo.tile([P, OUT], f32, tag="o2", name=f"o2_{w}")
                    nc.scalar.activation(
                        o2[:], pn2[:, 0:OUT], AF.Copy, scale=rden[:, 0:1])
                    if b2_nonzero:
                        nc.vector.tensor_tensor(
                            out=o2[:], in0=o2[:], in1=b2_sb[:], op=OP.add)
                    mx = ppo.tile([P, 1], f32, tag="mx", name=f"mx{w}")
                    nc.vector.reduce_max(
                        out=mx[:], in_=o2[:], axis=mybir.AxisListType.X)
                    negm = ppo.tile([P, 1], f32, tag="negm", name=f"negm{w}")
                    nc.vector.tensor_scalar(
                        out=negm[:], in0=mx[:], scalar1=-1.0, scalar2=None,
                        op0=OP.mult)
                    ex = ppo.tile([P, OUT], f32, tag="ex", name=f"ex{w}")
                    ssum = ppo.tile([P, 1], f32, tag="ssum", name=f"ssum{w}")
                    nc.scalar.activation(
                        ex[:], o2[:], AF.Exp, bias=negm[:, 0:1],
                        accum_out=ssum[:, 0:1])
                    lns = ppo.tile([P, 1], f32, tag="lns", name=f"lns{w}")
                    nc.scalar.activation(lns[:], ssum[:], AF.Ln)
                    sh = ppo.tile([P, 1], f32, tag="sh", name=f"sh{w}")
                    nc.vector.tensor_tensor(
                        out=sh[:], in0=negm[:], in1=lns[:], op=OP.subtract)
                    outt = ppo.tile([P, OUT], f32, tag="outt", name=f"outt{w}")
                    nc.scalar.activation(
                        outt[:], o2[:], AF.Identity, bias=sh[:, 0:1])
                    r = rows_of(w)
                    nc.sync.dma_start(
                        out=out_t[w * P:w * P + r, :], in_=outt[:r, :])

        dram_pool.__exit__(None, None, None)

    nc.compile()
    return nc


# ------------------------------------------------------------------ driver


def make_in_maps(cfg: Cfg, inputs: dict):
    x = np.asarray(inputs["x"], np.float32)
    edge_index = np.asarray(inputs["edge_index"])
    W1 = np.asarray(inputs["W1"], np.float32)
    a_src1 = np.asarray(inputs["a_src1"], np.float32)
    a_dst1 = np.asarray(inputs["a_dst1"], np.float32)
    b1 = np.asarray(inputs["b1"], np.float32)
    W2 = np.asarray(inputs["W2"], np.float32)
    a_src2 = np.asarray(inputs["a_src2"], np.float32)
    a_dst2 = np.asarray(inputs["a_dst2"], np.float32)
    b2 = np.asarray(inputs["b2"], np.float32)

    nch, TSC_c, eidx, edl = _schedule(cfg, edge_index)
    W1aug, W2aug = _prep_weights(cfg, W1, a_src1, a_dst1, W2, a_src2, a_dst2)
    iota = np.tile(np.arange(P, dtype=BF16), (P, 1))
    ident = np.eye(P, dtype=BF16)
    b1_nonzero = bool(np.any(b1))
    b2_nonzero = bool(np.any(b2))
    perm = _perm(cfg)

    NT = cfg.NWIN
    XPAD = NT * P
    xtab = x.astype(BF16)
    in_maps = []
    for k in range(cfg.NC):
        xs = x[k * cfg.NSH:(k + 1) * cfg.NSH]
        xTp = np.zeros((cfg.F, XPAD), BF16)
        xTp[:, :cfg.NSH] = xs.T.astype(BF16)
        m = {
            "xT": xTp,
            "xtab": xtab,
            "W1aug": W1aug,
            "W2aug": W2aug,
            "iota": iota,
            "ident": ident,
            "eidx": eidx[k],
            "edl": edl[k],
        }
        if b1_nonzero:
            m["b1rep"] = np.tile(b1[perm][None, :], (P, 1)).astype(np.float32)
        if b2_nonzero:
            m["b2rep"] = np.tile(b2[None, :], (P, 1)).astype(np.float32)
        in_maps.append(m)
    return in_maps, nch, TSC_c, b1_nonzero, b2_nonzero


class Executor:
    """Compile once; execute repeatedly through one jitted shard_map."""

    def __init__(self, cfg: Cfg, nch, TSC_c, b1nz, b2nz, mode="full"):
        import jax
        from jax.sharding import Mesh, PartitionSpec
        from jax.experimental.shard_map import shard_map
        from concourse import bass2jax
        import concourse.mybir as mybir_

        self.cfg = cfg
        nc = _build(cfg, nch, TSC_c, b1nz, b2nz, mode=mode)
        self.nc = nc
        bass2jax.install_neuronx_cc_hook()

        in_names, out_names, out_avals, zero_shapes = [], [], [], []
        for alloc in nc.m.functions[0].allocations:
            if not isinstance(alloc, mybir_.MemoryLocationSet):
                continue
            name = alloc.memorylocations[0].name
            if alloc.kind == "ExternalInput":
                in_names.append(name)
            elif alloc.kind == "ExternalOutput":
                shape = tuple(alloc.tensor_shape)
                dtype = mybir_.dt.np(alloc.dtype)
                out_avals.append(jax.core.ShapedArray(shape, dtype))
                out_names.append(name)
                zero_shapes.append((shape, dtype))
        assert nc.dbg_addr is None
        part_name = (nc.partition_id_tensor.name
                     if nc.partition_id_tensor else None)
        in_names = [n for n in in_names if n != part_name]
        n_params = len(in_names)
        all_names = in_names + out_names
        if part_name is not None:
            all_names = all_names + [part_name]
        donate = tuple(range(n_params, n_params + len(out_names)))

        def _body(*args):
            operands = list(args)
            if part_name is not None:
                operands.append(bass2jax.partition_id_tensor())
            outs = bass2jax._bass_exec_p.bind(
                *operands,
                out_avals=tuple(out_avals),
                in_names=tuple(all_names),
                out_names=tuple(out_names),
                lowering_input_output_aliases=(),
                sim_require_finite=True,
                sim_require_nnan=True,
                nc=nc,
            )
            return tuple(outs)

        devices = jax.devices()[:cfg.NC]
        mesh = Mesh(np.asarray(devices), ("core",))
        nio = n_params + len(out_names)
        self._fn = jax.jit(
            shard_map(
                _body, mesh=mesh,
                in_specs=(PartitionSpec("core"),) * nio,
                out_specs=(PartitionSpec("core"),) * len(out_names),
                check_rep=False,
            ),
            donate_argnums=donate, keep_unused=True,
        )
        self.in_names = in_names
        self.out_names = out_names
        self.out_avals = out_avals
        self.zero_shapes = zero_shapes
        self.mesh = mesh

    def bench(self, in_maps, iters=10):
        """Device-resident repeat timing: inputs uploaded once, outputs
        chained through donation. Returns per-iteration wall seconds."""
        import time
        import jax
        from jax.sharding import NamedSharding, PartitionSpec
        cfg = self.cfg
        sh = NamedSharding(self.mesh, PartitionSpec("core"))
        din = [
            jax.device_put(
                np.concatenate([np.asarray(m[n]) for m in in_maps], axis=0),
                sh)
            for n in self.in_names
        ]
        prev = [
            jax.device_put(np.zeros((cfg.NC * s[0], *s[1:]), d), sh)
            for s, d in self.zero_shapes
        ]
        outs = self._fn(*din, *prev)  # warm
        jax.block_until_ready(outs)
        times = []
        for _ in range(iters):
            t0 = time.perf_counter()
            outs = self._fn(*din, *outs)
            jax.block_until_ready(outs)
            times.append(time.perf_counter() - t0)
        return times

    def __call__(self, in_maps):
        cfg = self.cfg
        concat_in = [
            np.concatenate([np.asarray(m[n]) for m in in_maps], axis=0)
            for n in self.in_names
        ]
        concat_zeros = [
            np.zeros((cfg.NC * s[0], *s[1:]), d) for s, d in self.zero_shapes
        ]
        outs = self._fn(*concat_in, *concat_zeros)
        outs = [np.asarray(o) for o in outs]
        return [
            {
                n: outs[i].reshape(cfg.NC, *self.out_avals[i].shape)[c]
                for i, n in enumerate(self.out_names)
            }
            for c in range(cfg.NC)
        ]


_exec_cache = {}


def get_executor(cfg: Cfg, inputs: dict, mode="full"):
    in_maps, nch, TSC_c, b1nz, b2nz = make_in_maps(cfg, inputs)
    key = (cfg, tuple(tuple(c) for c in nch), b1nz, b2nz, mode)
    if key not in _exec_cache:
        _exec_cache[key] = Executor(cfg, nch, TSC_c, b1nz, b2nz, mode=mode)
    return _exec_cache[key], in_maps


def run(cfg: Cfg, inputs: dict, trace: bool = False):
    ex, in_maps = get_executor(cfg, inputs)
    results = ex(in_maps)
    out = np.concatenate([results[k]["out"] for k in range(cfg.NC)], 0)
    return out, ex


def kernel(**inputs) -> np.ndarray:
    out, _ = run(FULL, inputs)
    return out.astype(np.float32)


# revision 39
# speedup vs baseline: 1.0611x; 1.0597x over previous
"""GAT (2-layer graph attention network) on 8 Trainium2 NeuronCores.

Strategy (dst-sharded graph parallel, gather-x-recompute):
  - Nodes are partitioned across the 8 cores (6250 dst nodes each).
  - Host pre-sorts edges (incl. self-loops) by destination, groups them into
    128-dst "windows" and 128-edge "chunks", padding so every core runs an
    identical static program.  Chunks are segregated by source half (int16
    gather-index range), and batched into 2048-edge superchunks.
  - Layer 1 avoids any feature AllGather: each core holds the FULL x as a
    gather table; per superchunk one transposed dma_gather fetches raw x rows
    (512B each) feature-major, and h|alpha_src are recomputed PER EDGE on the
    tensor engine (x_edge^T @ W1aug) - the PE is otherwise idle and this cuts
    gather descriptors/bytes ~3x vs gathering precomputed h rows.
  - alpha_dst needs no gather at all: a_dst lives in an SBUF table indexed by
    (window, slot); the per-chunk one-hot is transposed on the PE and a tiny
    matmul selects per-edge a_dst.
  - Scatter-add into PSUM via one-hot matmuls per 128-dst window; softmax
    denominators from a second matmul against the same one-hot.
  - Layer 2: h2 rows are dst-core-local, so a small AllGather ([N,128] bf16)
    replicates them; per-edge rows gathered as before; a_dst2 via the same
    one-hot-transpose trick; then log_softmax.
"""

import math
from dataclasses import dataclass

import ml_dtypes
import numpy as np

import concourse.bass as bass
import concourse.mybir as mybir
import concourse.tile as tile
from concourse import bacc

BF16 = ml_dtypes.bfloat16
P = 128
NEG_SLOPE = 0.2
HALF = 25000  # src-table split point (int16 index range)


@dataclass(frozen=True)
class Cfg:
    N: int = 50000
    F: int = 256
    H: int = 8
    C: int = 64
    OUT: int = 40
    NC: int = 8
    SCK: int = 32          # chunks per gather superchunk

    @property
    def HC(self):
        return self.H * self.C

    @property
    def NSH(self):
        return self.N // self.NC

    @property
    def NWIN(self):
        return math.ceil(self.NSH / P)

    @property
    def nclass(self):
        return 2 if self.N > 32767 else 1


FULL = Cfg()


# ---------------------------------------------------------------- host side


def _schedule(cfg: Cfg, edge_index: np.ndarray):
    """Sort/pad edges into the common static chunk schedule.

    Returns (nch, TSC_c, eidx, edl):
      nch  : [nclass][NWIN] chunks per (class, window), common to all cores
      eidx : int16 [NC, TSC, 128, SCK*8]  src local-to-table indices
             (idx i of a superchunk at [i % 16 (+16g replicas), i // 16])
      edl  : float32 [NC, TSC, 128, SCK] dst-in-window (pad -1)
    """
    N, NCOR, NSH, SCK = cfg.N, cfg.NC, cfg.NSH, cfg.SCK
    NWIN, NCLS = cfg.NWIN, cfg.nclass
    # self-loops are handled on-chip per window (identity one-hot, local x),
    # so only the real edges enter the gathered chunk stream
    src = np.asarray(edge_index[0]).astype(np.int64)
    dst = np.asarray(edge_index[1]).astype(np.int64)
    core = (dst // NSH).astype(np.int32)
    dl = (dst % NSH).astype(np.int32)
    win = dl // P
    wloc = dl % P
    cls = (src >= HALF).astype(np.int32) if NCLS == 2 else np.zeros_like(core)

    counts = np.zeros((NCOR, NCLS, NWIN), np.int64)
    np.add.at(counts, (core, cls, win), 1)
    nch = np.maximum(np.ceil(counts / P).astype(np.int64).max(axis=0), 1)
    T_c = [int(nch[c].sum()) for c in range(NCLS)]
    TSC_c = [math.ceil(t / SCK) for t in T_c]
    TSC = sum(TSC_c)
    TP_c = [t * SCK for t in TSC_c]

    # slot offset of (class, window) inside its class stream (in edges)
    wstart = np.zeros((NCLS, NWIN + 1), np.int64)
    for c in range(NCLS):
        wstart[c, 1:] = np.cumsum(nch[c])
    wstart *= P

    eidx = np.zeros((NCOR, TSC, P, SCK * 8), np.int16)
    edlA = np.full((NCOR, TSC, P, SCK), -1.0, np.float32)
    sc_base = [0, TSC_c[0]] if NCLS == 2 else [0]

    for k in range(NCOR):
        for c in range(NCLS):
            m = (core == k) & (cls == c)
            s_k = src[m] - (HALF if c == 1 else 0)
            w_k = win[m]
            wl_k = wloc[m]
            order = np.lexsort((s_k, w_k))
            s_k, w_k, wl_k = s_k[order], w_k[order], wl_k[order]
            cnts = counts[k, c]
            offs = np.concatenate([[0], np.cumsum(cnts[:-1])])
            slot = wstart[c, w_k] + (np.arange(len(s_k)) - offs[w_k])
            es = np.zeros(TP_c[c] * P, np.int16)
            el = np.full(TP_c[c] * P, -1.0, np.float32)
            es[slot] = s_k.astype(np.int16)
            el[slot] = wl_k.astype(np.float32)
            # flat i -> idx tile [i%16, i//16] (replicated), edl [i%128, i//128]
            for si in range(TSC_c[c]):
                sc = sc_base[c] + si
                seg_s = es[si * SCK * P:(si + 1) * SCK * P]
                seg_l = el[si * SCK * P:(si + 1) * SCK * P]
                t16s = seg_s.reshape(-1, 16).T  # [16, SCK*8]
                eidx[k, sc, :, :] = np.tile(t16s, (8, 1))
                edlA[k, sc] = seg_l.reshape(SCK, P).T
    nch_py = [[int(x) for x in nch[c]] for c in range(NCLS)]
    return nch_py, TSC_c, eidx, edlA


def _perm(cfg: Cfg):
    p = np.empty(cfg.HC, np.int64)
    for h in range(cfg.H):
        p[np.arange(cfg.C) * cfg.H + h] = h * cfg.C + np.arange(cfg.C)
    return p


def _prep_weights(cfg: Cfg, W1, a_src1, a_dst1, W2, a_src2, a_dst2):
    perm = _perm(cfg)
    H, C, HC, OUT = cfg.H, cfg.C, cfg.HC, cfg.OUT
    Ws1 = np.stack([W1[:, h * C:(h + 1) * C] @ a_src1[h] for h in range(H)], 1)
    Wd1 = np.stack([W1[:, h * C:(h + 1) * C] @ a_dst1[h] for h in range(H)], 1)
    W1aug = np.concatenate([W1[:, perm], Ws1, Wd1], axis=1).astype(BF16)
    w2s = (W2 @ a_src2[0])[:, None]
    w2d = (W2 @ a_dst2[0])[:, None]
    L2PAD = 48 - (OUT + 2)
    W2aug = np.concatenate(
        [W2, w2s, w2d, np.zeros((HC, L2PAD), W2.dtype)], axis=1
    )[perm, :].astype(BF16)
    return W1aug, W2aug


# -------------------------------------------------------------- device side


def _build(cfg: Cfg, nch, TSC_c, b1_nonzero=False, b2_nonzero=False,
           mode="full"):
    N, F, H, C, HC, OUT = cfg.N, cfg.F, cfg.H, cfg.C, cfg.HC, cfg.OUT
    NSH, SCK, NWIN = cfg.NSH, cfg.SCK, cfg.NWIN
    NCLS = cfg.nclass
    TSC = sum(TSC_c)
    KT = math.ceil(F / P)
    BT = math.ceil(HC / P)
    AUG1 = HC + 2 * H
    AUG2 = 48
    ADW = 128                    # hx2 table row stride (elements)
    NT = NWIN
    XPAD = NT * P
    NIDX = SCK * P

    bf = mybir.dt.bfloat16
    f32 = mybir.dt.float32
    i16 = mybir.dt.int16
    AF = mybir.ActivationFunctionType
    OP = mybir.AluOpType

    nc = bacc.Bacc(
        "TRN2", target_bir_lowering=False, debug=False,
        enable_asserts=False, num_devices=cfg.NC,
    )

    xT_t = nc.dram_tensor("xT", [F, XPAD], bf, kind="ExternalInput")
    xtab_t = nc.dram_tensor("xtab", [N, F], bf, kind="ExternalInput")
    w1_t = nc.dram_tensor("W1aug", [F, AUG1], bf, kind="ExternalInput")
    w2_t = nc.dram_tensor("W2aug", [HC, AUG2], bf, kind="ExternalInput")
    iota_t = nc.dram_tensor("iota", [P, P], bf, kind="ExternalInput")
    ident_t = nc.dram_tensor("ident", [P, P], bf, kind="ExternalInput")
    eidx_t = nc.dram_tensor("eidx", [TSC, P, SCK * 8], i16,
                            kind="ExternalInput")
    edl_t = nc.dram_tensor("edl", [TSC, P, SCK], f32, kind="ExternalInput")
    if b1_nonzero:
        b1_t = nc.dram_tensor("b1rep", [P, HC], f32, kind="ExternalInput")
    if b2_nonzero:
        b2_t = nc.dram_tensor("b2rep", [P, OUT], f32, kind="ExternalInput")
    out_t = nc.dram_tensor("out", [NSH, OUT], f32, kind="ExternalOutput")

    # (class, sc, kk) schedule per window: class streams are contiguous
    sc_base = [0, TSC_c[0]] if NCLS == 2 else [0]
    window_chunks = []  # per window: list of (c, sc, kk)
    pos_c = [0] * NCLS
    for w in range(NWIN):
        lst = []
        for c in range(NCLS):
            for _ in range(nch[c][w]):
                j = pos_c[c]
                lst.append((c, sc_base[c] + j // SCK, j % SCK))
                pos_c[c] += 1
        window_chunks.append(lst)
    rows_of = lambda w: min(P, NSH - w * P)
    # class of each sc (for table selection) and window of each (sc, kk)
    sc_cls = [0] * TSC
    if NCLS == 2:
        for s in range(TSC_c[0], TSC):
            sc_cls[s] = 1
    win_of = [[0] * SCK for _ in range(TSC)]
    for w in range(NWIN):
        for (c, sc, kk) in window_chunks[w]:
            win_of[sc][kk] = w

    with tile.TileContext(nc) as tc:
        dram_pool = tc.tile_pool(name="dram", bufs=1, space="DRAM")
        pdr = dram_pool.__enter__()
        shared_as = "Shared" if cfg.NC > 4 else "Local"
        hx2_dram = pdr.tile([NSH, ADW], bf, name="hx2_dram")
        hx2_full = pdr.tile([N, ADW], bf, addr_space=shared_as,
                            name="hx2_full")

        def xtab_src(sc):
            if NCLS == 1 or sc_cls[sc] == 0:
                return xtab_t[0:min(HALF, N), :]
            return xtab_t[HALF:N, :]

        def hx2_src(sc):
            if NCLS == 1 or sc_cls[sc] == 0:
                return hx2_full[0:min(HALF, N), :]
            return hx2_full[HALF:N, :]

        with tc.tile_pool(name="const", bufs=1) as pc:
            iota_sb = pc.tile([P, P], bf, name="iota_sb")
            nc.sync.dma_start(out=iota_sb[:], in_=iota_t[:, :])
            ident_sb = pc.tile([P, P], bf, name="ident_sb")
            nc.sync.dma_start(out=ident_sb[:], in_=ident_t[:, :])
            w1_sb = []
            for kk in range(KT):
                r = min(P, F - kk * P)
                t_ = pc.tile([r, AUG1], bf, name=f"w1_sb{kk}")
                nc.sync.dma_start(out=t_[:], in_=w1_t[kk * P:kk * P + r, :])
                w1_sb.append(t_)
            w2_sb = []
            for b in range(BT):
                r = min(P, HC - b * P)
                t_ = pc.tile([r, AUG2], bf, name=f"w2_sb{b}")
                nc.sync.dma_start(out=t_[:], in_=w2_t[b * P:b * P + r, :])
                w2_sb.append(t_)
            xT_sb = []
            for kk in range(KT):
                r = min(P, F - kk * P)
                t_ = pc.tile([r, XPAD], bf, name=f"xT_sb{kk}")
                nc.sync.dma_start(out=t_[:], in_=xT_t[kk * P:kk * P + r, :])
                xT_sb.append(t_)
            if b1_nonzero:
                b1_sb = pc.tile([P, HC], f32, name="b1_sb")
                nc.sync.dma_start(out=b1_sb[:], in_=b1_t[:, :])
            if b2_nonzero:
                b2_sb = pc.tile([P, OUT], f32, name="b2_sb")
                nc.sync.dma_start(out=b2_sb[:], in_=b2_t[:, :])
            # SBUF a_dst table: [slot, window, head(8) + l2(1)]
            adt_sb = pc.tile([P, NWIN, H + 1], bf, name="adt_sb")
            # SBUF copy of this core's own hx2 rows (for layer-2 self-loops)
            hx2sb = pc.tile([P, NWIN, AUG2], bf, name="hx2sb")

            # ---------------- phase A: a_dst table = (x @ Wd1) per window
            with tc.tile_pool(name="phA_ps", bufs=2, space="PSUM") as pap:
                for t in range(NT):
                    pa = pap.tile([P, 2 * H], f32, tag="pA", name=f"pA{t}")
                    for kk in range(KT):
                        lhsT = xT_sb[kk][:, t * P:(t + 1) * P]
                        nc.tensor.matmul(
                            out=pa[:], lhsT=lhsT, rhs=w1_sb[kk][:, HC:AUG1],
                            start=(kk == 0), stop=(kk == KT - 1))
                    nc.vector.tensor_copy(
                        out=adt_sb[:, t, 0:H], in_=pa[:, H:2 * H])

            # ---------------- phase C: layer-1 edge loop (+ h2 per window)
            with tc.tile_pool(name="phC_st", bufs=2) as pst, \
                 tc.tile_pool(name="phC_ck", bufs=4) as pck, \
                 tc.tile_pool(name="phC_po", bufs=2) as ppo, \
                 tc.tile_pool(name="phC_he", bufs=1, space="PSUM") as phe, \
                 tc.tile_pool(name="phC_ps", bufs=1, space="PSUM") as pps, \
                 tc.tile_pool(name="phC_sp", bufs=1, space="PSUM") as psp, \
                 tc.tile_pool(name="phC_pt", bufs=1, space="PSUM") as ppt:

                sc_cache = {}

                def get_sc(sc):
                    if sc in sc_cache:
                        return sc_cache[sc]
                    six = pst.tile([P, SCK * 8], i16, tag="six",
                                   name=f"six{sc}")
                    nc.sync.dma_start(out=six[:], in_=eidx_t[sc, :, :])
                    edl = pst.tile([P, SCK], f32, tag="edl", name=f"edl{sc}")
                    nc.sync.dma_start(out=edl[:], in_=edl_t[sc, :, :])
                    xg = pst.tile([P, KT, NIDX], bf, tag="xg", name=f"xg{sc}")
                    nc.gpsimd.dma_gather(
                        xg[:], xtab_src(sc), six[:],
                        NIDX, NIDX, F, transpose=True, single_packet=False)
                    ohs = pst.tile([P, SCK, P], bf, tag="ohs", name=f"ohs{sc}")
                    ohT = pst.tile([P, SCK, P], bf, tag="ohT", name=f"ohT{sc}")
                    adsp = psp.tile([P, SCK, H], f32, tag="adsp",
                                    name=f"adsp{sc}")
                    for kk in range(SCK):
                        nc.vector.tensor_scalar(
                            out=ohs[:, kk, :], in0=iota_sb[:],
                            scalar1=edl[:, kk:kk + 1], scalar2=None,
                            op0=OP.is_equal)
                        tp = ppt.tile([P, P], bf, tag="tp",
                                      name=f"tp{sc}_{kk}")
                        nc.tensor.transpose(
                            out=tp[:], in_=ohs[:, kk, :],
                            identity=ident_sb[:])
                        nc.scalar.activation(ohT[:, kk, :], tp[:], AF.Copy)
                        # a_src(x_e) and one-hot-selected a_dst accumulate
                        # into the same PSUM region: es_raw = x_e.Ws1 + a_dst
                        for kt in range(KT):
                            nc.tensor.matmul(
                                out=adsp[:, kk, :],
                                lhsT=xg[:, kt, kk * P:(kk + 1) * P],
                                rhs=w1_sb[kt][:, HC:HC + H],
                                start=(kt == 0), stop=False)
                        nc.tensor.matmul(
                            out=adsp[:, kk, :], lhsT=ohT[:, kk, :],
                            rhs=adt_sb[:, win_of[sc][kk], 0:H],
                            start=False, stop=True)
                    es = pst.tile([P, SCK, H], f32, tag="es", name=f"es{sc}")
                    nc.vector.tensor_copy(out=es[:], in_=adsp[:, :, :])
                    elr = pst.tile([P, SCK, H], f32, tag="elr",
                                   name=f"elr{sc}")
                    nc.vector.scalar_tensor_tensor(
                        out=elr[:], in0=es[:], scalar=NEG_SLOPE, in1=es[:],
                        op0=OP.mult, op1=OP.max)
                    wts = pst.tile([P, SCK, H], bf, tag="wts", name=f"wts{sc}")
                    nc.scalar.activation(wts[:], elr[:], AF.Exp)
                    sc_cache[sc] = (xg, ohs, wts)
                    return sc_cache[sc]

                if mode == "ABG":
                    for sc in range(TSC):
                        get_sc(sc)
                for w in range(NWIN if mode in ("full", "NOCC") else 0):
                    pn1 = pps.tile([P, HC], f32, tag="pn1", name=f"pn1_{w}")
                    aux = pps.tile([P, 2 * H + AUG2], f32, tag="aux",
                                   name=f"aux{w}")
                    pd1 = aux[:, 0:H]
                    njw = len(window_chunks[w])
                    for i, (c, sc, kk) in enumerate(window_chunks[w]):
                        xg, ohs, wts = get_sc(sc)
                        he = phe.tile([P, HC], f32, tag="he",
                                      name=f"he{w}_{i}")
                        for kt in range(KT):
                            nc.tensor.matmul(
                                out=he[:],
                                lhsT=xg[:, kt, kk * P:(kk + 1) * P],
                                rhs=w1_sb[kt][:, 0:HC],
                                start=(kt == 0), stop=(kt == KT - 1))
                        hesb = pck.tile([P, HC], bf, tag="hesb",
                                        name=f"hesb{w}_{i}")
                        nc.scalar.activation(hesb[:], he[:], AF.Copy)
                        msg = pck.tile([P, HC], bf, tag="msg",
                                       name=f"msg{w}_{i}")
                        nc.vector.tensor_tensor(
                            out=msg[:].rearrange("p (c h) -> p c h", h=H),
                            in0=hesb[:].rearrange("p (c h) -> p c h", h=H),
                            in1=wts[:, kk:kk + 1, :].to_broadcast([P, C, H]),
                            op=OP.mult)
                        nc.tensor.matmul(
                            out=pn1[:], lhsT=ohs[:, kk, :], rhs=msg[:],
                            start=(i == 0), stop=False)
                        nc.tensor.matmul(
                            out=pd1, lhsT=ohs[:, kk, :], rhs=wts[:, kk, :],
                            start=(i == 0), stop=False)

                    # self-loop chunk: x rows of this window live in xT_sb,
                    # one-hot is the identity, a_dst read straight from adt_sb
                    hes = phe.tile([P, HC], f32, tag="he", name=f"hes{w}")
                    for kt in range(KT):
                        nc.tensor.matmul(
                            out=hes[:], lhsT=xT_sb[kt][:, w * P:(w + 1) * P],
                            rhs=w1_sb[kt][:, 0:HC],
                            start=(kt == 0), stop=(kt == KT - 1))
                    assl = psp.tile([P, H], f32, tag="assl", name=f"assl{w}")
                    for kt in range(KT):
                        nc.tensor.matmul(
                            out=assl[:],
                            lhsT=xT_sb[kt][:, w * P:(w + 1) * P],
                            rhs=w1_sb[kt][:, HC:HC + H],
                            start=(kt == 0), stop=False)
                    nc.tensor.matmul(
                        out=assl[:], lhsT=ident_sb[:],
                        rhs=adt_sb[:, w, 0:H], start=False, stop=True)
                    hesbs = pck.tile([P, HC], bf, tag="hesb",
                                     name=f"hesbs{w}")
                    nc.scalar.activation(hesbs[:], hes[:], AF.Copy)
                    ess = ppo.tile([P, H], f32, tag="ess", name=f"ess{w}")
                    nc.vector.tensor_copy(out=ess[:], in_=assl[:])
                    elrs = ppo.tile([P, H], f32, tag="elrs", name=f"elrs{w}")
                    nc.vector.scalar_tensor_tensor(
                        out=elrs[:], in0=ess[:], scalar=NEG_SLOPE,
                        in1=ess[:], op0=OP.mult, op1=OP.max)
                    wtss = ppo.tile([P, 1, H], bf, tag="wtss",
                                    name=f"wtss{w}")
                    nc.scalar.activation(wtss[:, 0, :], elrs[:], AF.Exp)
                    msgs = pck.tile([P, HC], bf, tag="msg", name=f"msgs{w}")
                    nc.vector.tensor_tensor(
                        out=msgs[:].rearrange("p (c h) -> p c h", h=H),
                        in0=hesbs[:].rearrange("p (c h) -> p c h", h=H),
                        in1=wtss[:, :, :].to_broadcast([P, C, H]),
                        op=OP.mult)
                    nc.tensor.matmul(
                        out=pn1[:], lhsT=ident_sb[:], rhs=msgs[:],
                        start=False, stop=True)
                    nc.tensor.matmul(
                        out=pd1, lhsT=ident_sb[:], rhs=wtss[:, 0, :],
                        start=False, stop=True)

                    den = ppo.tile([P, H], f32, tag="den", name=f"den{w}")
                    nc.vector.tensor_scalar(
                        out=den[:], in0=pd1, scalar1=1e-30, scalar2=None,
                        op0=OP.add)
                    rden = ppo.tile([P, H], f32, tag="rden", name=f"rden{w}")
                    nc.vector.reciprocal(out=rden[:], in_=den[:])
                    h1a = ppo.tile([P, HC], bf, tag="h1a", name=f"h1a{w}")
                    h1v = h1a[:].rearrange("p (c h) -> p c h", h=H)
                    pnv = pn1[:].rearrange("p (c h) -> p c h", h=H)
                    if not b1_nonzero:
                        for h in range(H):
                            nc.scalar.activation(
                                h1v[:, :, h:h + 1], pnv[:, :, h:h + 1],
                                AF.Relu, scale=rden[:, h:h + 1])
                    else:
                        t1 = ppo.tile([P, HC], f32, tag="t1", name=f"t1_{w}")
                        t1v = t1[:].rearrange("p (c h) -> p c h", h=H)
                        for h in range(H):
                            nc.scalar.activation(
                                t1v[:, :, h:h + 1], pnv[:, :, h:h + 1],
                                AF.Copy, scale=rden[:, h:h + 1])
                        nc.vector.tensor_tensor(
                            out=t1[:], in0=t1[:], in1=b1_sb[:], op=OP.add)
                        nc.vector.tensor_scalar(
                            out=h1a[:], in0=t1[:], scalar1=0.0, scalar2=None,
                            op0=OP.max)

                    # layer-2 pre-pass for this node tile
                    ph2 = aux[:, 2 * H:2 * H + AUG2]
                    for b in range(BT):
                        r = min(P, HC - b * P)
                        tp = ppt.tile([P, P], bf, tag="tp2w",
                                      name=f"tpw{w}_{b}")
                        nc.tensor.transpose(
                            out=tp[:r, :], in_=h1a[:, b * P:b * P + r],
                            identity=ident_sb[:])
                        h1T = ppo.tile([P, P], bf, tag="h1T",
                                       name=f"h1T{w}_{b}")
                        nc.scalar.activation(h1T[:r, :], tp[:r, :], AF.Copy)
                        nc.tensor.matmul(
                            out=ph2, lhsT=h1T[:r, :], rhs=w2_sb[b][:],
                            start=(b == 0), stop=(b == BT - 1))
                    nc.scalar.activation(hx2sb[:, w, :], ph2, AF.Copy)
                    nc.vector.tensor_copy(
                        out=adt_sb[:, w, H:H + 1],
                        in_=aux[:, 2 * H + OUT + 1:2 * H + OUT + 2])
                    r = rows_of(w)
                    nc.sync.dma_start(
                        out=hx2_dram[w * P:w * P + r, 0:AUG2],
                        in_=hx2sb[:r, w, :])

            # ---------------- AllGather layer-2 features
            if mode == "NOCC":
                nc.sync.dma_start(out=hx2_full[0:NSH, :],
                                  in_=hx2_dram[0:NSH, :])
            elif mode != "A":
                nc.gpsimd.collective_compute(
                    "AllGather", OP.bypass,
                    replica_groups=[list(range(cfg.NC))],
                    ins=[hx2_dram.opt()], outs=[hx2_full.opt()],
                )

            # ---------------- phase D: layer-2 edge loop + log_softmax
            with tc.tile_pool(name="phD_st", bufs=2) as pst, \
                 tc.tile_pool(name="phD_ck", bufs=4) as pck, \
                 tc.tile_pool(name="phD_po", bufs=2) as ppo, \
                 tc.tile_pool(name="phD_ps", bufs=2, space="PSUM") as pps, \
                 tc.tile_pool(name="phD_sp", bufs=1, space="PSUM") as psp, \
                 tc.tile_pool(name="phD_pt", bufs=2, space="PSUM") as ppt:

                sc2_cache = {}

                def get_sc2(sc):
                    if sc in sc2_cache:
                        return sc2_cache[sc]
                    six = pst.tile([P, SCK * 8], i16, tag="six2",
                                   name=f"s2ix{sc}")
                    nc.sync.dma_start(out=six[:], in_=eidx_t[sc, :, :])
                    edl = pst.tile([P, SCK], f32, tag="edl2",
                                   name=f"edl2_{sc}")
                    nc.sync.dma_start(out=edl[:], in_=edl_t[sc, :, :])
                    hxg = pst.tile([P, SCK, ADW], bf, tag="hxg2",
                                   name=f"hxg2_{sc}")
                    nc.gpsimd.dma_gather(
                        hxg[:], hx2_src(sc), six[:],
                        NIDX, NIDX, ADW, single_packet=False)
                    ohs = pst.tile([P, SCK, P], bf, tag="ohs2",
                                   name=f"ohs2_{sc}")
                    ohT = pst.tile([P, SCK, P], bf, tag="ohT2",
                                   name=f"ohT2_{sc}")
                    ade = psp.tile([P, SCK, 1], f32, tag="ade2",
                                   name=f"ade2_{sc}")
                    for kk in range(SCK):
                        nc.vector.tensor_scalar(
                            out=ohs[:, kk, :], in0=iota_sb[:],
                            scalar1=edl[:, kk:kk + 1], scalar2=None,
                            op0=OP.is_equal)
                        tp = ppt.tile([P, P], bf, tag="tp2",
                                      name=f"tp2_{sc}_{kk}")
                        nc.tensor.transpose(
                            out=tp[:], in_=ohs[:, kk, :],
                            identity=ident_sb[:])
                        nc.scalar.activation(ohT[:, kk, :], tp[:], AF.Copy)
                        nc.tensor.matmul(
                            out=ade[:, kk, :], lhsT=ohT[:, kk, :],
                            rhs=adt_sb[:, win_of[sc][kk], H:H + 1],
                            start=True, stop=True)
                    adesb = pst.tile([P, SCK, 1], f32, tag="adesb2",
                                     name=f"adesb2_{sc}")
                    nc.vector.tensor_copy(out=adesb[:], in_=ade[:, :, :])
                    es = pst.tile([P, SCK, 1], f32, tag="es2",
                                  name=f"es2_{sc}")
                    nc.vector.tensor_tensor(
                        out=es[:], in0=hxg[:, :, OUT:OUT + 1],
                        in1=adesb[:], op=OP.add)
                    elr = pst.tile([P, SCK, 1], f32, tag="elr2",
                                   name=f"elr2_{sc}")
                    nc.vector.scalar_tensor_tensor(
                        out=elr[:], in0=es[:], scalar=NEG_SLOPE, in1=es[:],
                        op0=OP.mult, op1=OP.max)
                    wts = pst.tile([P, SCK, 1], f32, tag="wts2",
                                   name=f"wts2_{sc}")
                    nc.scalar.activation(wts[:], elr[:], AF.Exp)
                    wtsb = pst.tile([P, SCK, 1], bf, tag="wtsb2",
                                    name=f"wtsb2_{sc}")
                    nc.vector.tensor_copy(out=wtsb[:], in_=wts[:])
                    sc2_cache[sc] = (hxg, ohs, wts, wtsb)
                    return sc2_cache[sc]

                if mode == "ABG":
                    for sc in range(TSC):
                        get_sc2(sc)
                if mode in ("A", "AB", "ABG"):
                    zt = ppo.tile([P, OUT], f32, tag="zt", name="zt")
                    nc.vector.memset(zt[:], 0.0)
                    for w in range(NWIN):
                        r = rows_of(w)
                        nc.sync.dma_start(
                            out=out_t[w * P:w * P + r, :], in_=zt[:r, :])
                for w in range(NWIN if mode not in ("A", "AB", "ABG") else 0):
                    pn2 = pps.tile([P, OUT], f32, tag="pn2", name=f"pn2_{w}")
                    pd2 = pps.tile([P, 1], f32, tag="pd2", name=f"pd2_{w}")
                    njw = len(window_chunks[w])
                    for i, (c, sc, kk) in enumerate(window_chunks[w]):
                        hxg, ohs, wts, wtsb = get_sc2(sc)
                        msg = pck.tile([P, OUT], bf, tag="msg2",
                                       name=f"ms2{w}_{i}")
                        nc.vector.tensor_scalar(
                            out=msg[:], in0=hxg[:, kk, 0:OUT],
                            scalar1=wts[:, kk, :], scalar2=None, op0=OP.mult)
                        nc.tensor.matmul(
                            out=pn2[:], lhsT=ohs[:, kk, :], rhs=msg[:],
                            start=(i == 0), stop=False)
                        nc.tensor.matmul(
                            out=pd2[:], lhsT=ohs[:, kk, :], rhs=wtsb[:, kk, :],
                            start=(i == 0), stop=False)

                    # layer-2 self-loop chunk from the SBUF-resident own rows
                    es2s = ppo.tile([P, 1], f32, tag="es2s", name=f"es2s{w}")
                    nc.vector.tensor_tensor(
                        out=es2s[:], in0=hx2sb[:, w, OUT:OUT + 1],
                        in1=adt_sb[:, w, H:H + 1], op=OP.add)
                    elr2s = ppo.tile([P, 1], f32, tag="elr2s",
                                     name=f"elr2s{w}")
                    nc.vector.scalar_tensor_tensor(
                        out=elr2s[:], in0=es2s[:], scalar=NEG_SLOPE,
                        in1=es2s[:], op0=OP.mult, op1=OP.max)
                    wt2s = ppo.tile([P, 1], f32, tag="wt2s", name=f"wt2s{w}")
                    nc.scalar.activation(wt2s[:], elr2s[:], AF.Exp)
                    wt2sb = ppo.tile([P, 1], bf, tag="wt2sb",
                                     name=f"wt2sb{w}")
                    nc.vector.tensor_copy(out=wt2sb[:], in_=wt2s[:])
                    msg2s = pck.tile([P, OUT], bf, tag="msg2",
                                     name=f"msg2s{w}")
                    nc.vector.tensor_scalar(
                        out=msg2s[:], in0=hx2sb[:, w, 0:OUT],
                        scalar1=wt2s[:, :], scalar2=None, op0=OP.mult)
                    nc.tensor.matmul(
                        out=pn2[:], lhsT=ident_sb[:], rhs=msg2s[:],
                        start=False, stop=True)
                    nc.tensor.matmul(
                        out=pd2[:], lhsT=ident_sb[:], rhs=wt2sb[:],
                        start=False, stop=True)

                    den = ppo.tile([P, 1], f32, tag="den2", name=f"den2_{w}")
                    nc.vector.tensor_scalar(
                        out=den[:], in0=pd2[:], scalar1=1e-30, scalar2=None,
                        op0=OP.add)
                    rden = ppo.tile([P, 1], f32, tag="rden2", name=f"rd2_{w}")
                    nc.vector.reciprocal(out=rden[:], in_=den[:])
                    o2 = ppo.tile([P, OUT], f32, tag="o2", name=f"o2_{w}")
                    nc.scalar.activation(
                        o2[:], pn2[:, 0:OUT], AF.Copy, scale=rden[:, 0:1])
                    if b2_nonzero:
                        nc.vector.tensor_tensor(
                            out=o2[:], in0=o2[:], in1=b2_sb[:], op=OP.add)
                    mx = ppo.tile([P, 1], f32, tag="mx", name=f"mx{w}")
                    nc.vector.reduce_max(
                        out=mx[:], in_=o2[:], axis=mybir.AxisListType.X)
                    negm = ppo.tile([P, 1], f32, tag="negm", name=f"negm{w}")
                    nc.vector.tensor_scalar(
                        out=negm[:], in0=mx[:], scalar1=-1.0, scalar2=None,
                        op0=OP.mult)
                    ex = ppo.tile([P, OUT], f32, tag="ex", name=f"ex{w}")
                    ssum = ppo.tile([P, 1], f32, tag="ssum", name=f"ssum{w}")
                    nc.scalar.activation(
                        ex[:], o2[:], AF.Exp, bias=negm[:, 0:1],
                        accum_out=ssum[:, 0:1])
                    lns = ppo.tile([P, 1], f32, tag="lns", name=f"lns{w}")
                    nc.scalar.activation(lns[:], ssum[:], AF.Ln)
                    sh = ppo.tile([P, 1], f32, tag="sh", name=f"sh{w}")
                    nc.vector.tensor_tensor(
                        out=sh[:], in0=negm[:], in1=lns[:], op=OP.subtract)
                    outt = ppo.tile([P, OUT], f32, tag="outt", name=f"outt{w}")
                    nc.scalar.activation(
                        outt[:], o2[:], AF.Identity, bias=sh[:, 0:1])
                    r = rows_of(w)
                    nc.sync.dma_start(
                        out=out_t[w * P:w * P + r, :], in_=outt[:r, :])

        dram_pool.__exit__(None, None, None)

    nc.compile()
    return nc


# ------------------------------------------------------------------ driver


def make_in_maps(cfg: Cfg, inputs: dict):
    x = np.asarray(inputs["x"], np.float32)
    edge_index = np.asarray(inputs["edge_index"])
    W1 = np.asarray(inputs["W1"], np.float32)
    a_src1 = np.asarray(inputs["a_src1"], np.float32)
    a_dst1 = np.asarray(inputs["a_dst1"], np.float32)
    b1 = np.asarray(inputs["b1"], np.float32)
    W2 = np.asarray(inputs["W2"], np.float32)
    a_src2 = np.asarray(inputs["a_src2"], np.float32)
    a_dst2 = np.asarray(inputs["a_dst2"], np.float32)
    b2 = np.asarray(inputs["b2"], np.float32)

    nch, TSC_c, eidx, edl = _schedule(cfg, edge_index)
    W1aug, W2aug = _prep_weights(cfg, W1, a_src1, a_dst1, W2, a_src2, a_dst2)
    iota = np.tile(np.arange(P, dtype=BF16), (P, 1))
    ident = np.eye(P, dtype=BF16)
    b1_nonzero = bool(np.any(b1))
    b2_nonzero = bool(np.any(b2))
    perm = _perm(cfg)

    NT = cfg.NWIN
    XPAD = NT * P
    xtab = x.astype(BF16)
    in_maps = []
    for k in range(cfg.NC):
        xs = x[k * cfg.NSH:(k + 1) * cfg.NSH]
        xTp = np.zeros((cfg.F, XPAD), BF16)
        xTp[:, :cfg.NSH] = xs.T.astype(BF16)
        m = {
            "xT": xTp,
            "xtab": xtab,
            "W1aug": W1aug,
            "W2aug": W2aug,
            "iota": iota,
            "ident": ident,
            "eidx": eidx[k],
            "edl": edl[k],
        }
        if b1_nonzero:
            m["b1rep"] = np.tile(b1[perm][None, :], (P, 1)).astype(np.float32)
        if b2_nonzero:
            m["b2rep"] = np.tile(b2[None, :], (P, 1)).astype(np.float32)
        in_maps.append(m)
    return in_maps, nch, TSC_c, b1_nonzero, b2_nonzero


class Executor:
    """Compile once; execute repeatedly through one jitted shard_map."""

    def __init__(self, cfg: Cfg, nch, TSC_c, b1nz, b2nz, mode="full"):
        import jax
        from jax.sharding import Mesh, PartitionSpec
        from jax.experimental.shard_map import shard_map
        from concourse import bass2jax
        import concourse.mybir as mybir_

        self.cfg = cfg
        nc = _build(cfg, nch, TSC_c, b1nz, b2nz, mode=mode)
        self.nc = nc
        bass2jax.install_neuronx_cc_hook()

        in_names, out_names, out_avals, zero_shapes = [], [], [], []
        for alloc in nc.m.functions[0].allocations:
            if not isinstance(alloc, mybir_.MemoryLocationSet):
                continue
            name = alloc.memorylocations[0].name
            if alloc.kind == "ExternalInput":
                in_names.append(name)
            elif alloc.kind == "ExternalOutput":
                shape = tuple(alloc.tensor_shape)
                dtype = mybir_.dt.np(alloc.dtype)
                out_avals.append(jax.core.ShapedArray(shape, dtype))
                out_names.append(name)
                zero_shapes.append((shape, dtype))
        assert nc.dbg_addr is None
        part_name = (nc.partition_id_tensor.name
                     if nc.partition_id_tensor else None)
        in_names = [n for n in in_names if n != part_name]
        n_params = len(in_names)
        all_names = in_names + out_names
        if part_name is not None:
            all_names = all_names + [part_name]
        donate = tuple(range(n_params, n_params + len(out_names)))

        def _body(*args):
            operands = list(args)
            if part_name is not None:
                operands.append(bass2jax.partition_id_tensor())
            outs = bass2jax._bass_exec_p.bind(
                *operands,
                out_avals=tuple(out_avals),
                in_names=tuple(all_names),
                out_names=tuple(out_names),
                lowering_input_output_aliases=(),
                sim_require_finite=True,
                sim_require_nnan=True,
                nc=nc,
            )
            return tuple(outs)

        devices = jax.devices()[:cfg.NC]
        mesh = Mesh(np.asarray(devices), ("core",))
        nio = n_params + len(out_names)
        self._fn = jax.jit(
            shard_map(
                _body, mesh=mesh,
                in_specs=(PartitionSpec("core"),) * nio,
                out_specs=(PartitionSpec("core"),) * len(out_names),
                check_rep=False,
            ),
            donate_argnums=donate, keep_unused=True,
        )
        self.in_names = in_names
        self.out_names = out_names
        self.out_avals = out_avals
        self.zero_shapes = zero_shapes
        self.mesh = mesh

    def bench(self, in_maps, iters=10):
        """Device-resident repeat timing: inputs uploaded once, outputs
        chained through donation. Returns per-iteration wall seconds."""
        import time
        import jax
        from jax.sharding import NamedSharding, PartitionSpec
        cfg = self.cfg
        sh = NamedSharding(self.mesh, PartitionSpec("core"))
        din = [
            jax.device_put(
                np.concatenate([np.asarray(m[n]) for m in in_maps], axis=0),
                sh)
            for n in self.in_names
        ]
        prev = [
            jax.device_put(np.zeros((cfg.NC * s[0], *s[1:]), d), sh)
            for s, d in self.zero_shapes
        ]
        outs = self._fn(*din, *prev)  # warm
        jax.block_until_ready(outs)
        times = []
        for _ in range(iters):
            t0 = time.perf_counter()
            outs = self._fn(*din, *outs)
            jax.block_until_ready(outs)
            times.append(time.perf_counter() - t0)
        return times

    def __call__(self, in_maps):
        cfg = self.cfg
        concat_in = [
            np.concatenate([np.asarray(m[n]) for m in in_maps], axis=0)
            for n in self.in_names
        ]
        concat_zeros = [
            np.zeros((cfg.NC * s[0], *s[1:]), d) for s, d in self.zero_shapes
        ]
        outs = self._fn(*concat_in, *concat_zeros)
        outs = [np.asarray(o) for o in outs]
        return [
            {
                n: outs[i].reshape(cfg.NC, *self.out_avals[i].shape)[c]
                for i, n in enumerate(self.out_names)
            }
            for c in range(cfg.NC)
        ]


_exec_cache = {}


def get_executor(cfg: Cfg, inputs: dict, mode="full"):
    in_maps, nch, TSC_c, b1nz, b2nz = make_in_maps(cfg, inputs)
    key = (cfg, tuple(tuple(c) for c in nch), b1nz, b2nz, mode)
    if key not in _exec_cache:
        _exec_cache[key] = Executor(cfg, nch, TSC_c, b1nz, b2nz, mode=mode)
    return _exec_cache[key], in_maps


def run(cfg: Cfg, inputs: dict, trace: bool = False):
    ex, in_maps = get_executor(cfg, inputs)
    results = ex(in_maps)
    out = np.concatenate([results[k]["out"] for k in range(cfg.NC)], 0)
    return out, ex


def kernel(**inputs) -> np.ndarray:
    out, _ = run(FULL, inputs)
    return out.astype(np.float32)
